# revision 2
# baseline (speedup 1.0000x reference)
# RWKV-v4 block (TimeMix WKV + ChannelMix) on 8 Trainium2 NeuronCores — v2.
#
# Sharding: data-parallel over the 16 (p, b) sequences -> 2 per core (both
# sequences of a core share the same p, so mix/fold constants are per-core).
#
# v2 strategy vs the bf16 baseline (924us cost-model):
# - All seven matmuls run as fp8e4m3 DoubleRow (2 k-tiles interleaved,
#   K=256/instr at 0.5 cyc/row): 4x fewer PE cycles than bf16.
# - TimeMix token-shift mixes are folded into the weights:
#   k = (Wk*diag(mk)) @ h + (Wk*diag(1-mk)) @ shift(h); h itself is written
#   directly in fp8 pair tiles [128, 2, TC+1] (col 0 = carry), so no mix
#   tiles and no shift ops exist on the vector engines for the k/v/r paths.
# - Weights are host-prescaled by WS=64 into the e4m3 normal range; every
#   inverse scale folds into an Activation scale= or the sigmoid chain.
# - WKV pipeline in bf16 (2x DVE mode on tensor_tensor), scans keep f32
#   lambda, reciprocals in f32 (hw requirement).
# - ChannelMix: cm_mix_k == cm_mix_r (runtime-checked) -> one shared mix
#   tile in fp8; Wck/Wcv are streamed per-unit in hb-/cb-blocked fp8 pair
#   layouts; relu on Act, square on Pool (gpsimd) -> fp8.
import os
import numpy as np
import ml_dtypes

P, B, T, C = 2, 8, 1024, 1024
H = 4 * C
NCORES = 8
NSEQ = 2
TC = 512
NCH = T // TC      # 2
CB = C // 128      # 8
PR = CB // 2       # 4 channel-block pairs
HB = H // 128      # 32
HPR = HB // 2      # 16
EPS = 1e-5
WS = 64.0          # fp8 weight prescale

_CACHE = {}
PHASES = []


def _build(use_gb1, use_gb2, debug=False):
    import concourse.bass as bass
    import concourse.tile as tile
    from concourse import bacc, mybir

    f32 = mybir.dt.float32
    f32r = mybir.dt.float32r
    bf16 = mybir.dt.bfloat16
    f8 = mybir.dt.float8e4
    AL = mybir.AluOpType
    AF = mybir.ActivationFunctionType
    PM = mybir.MatmulPerfMode

    nc = bacc.Bacc()
    global PHASES
    PHASES = []

    def mark(label):
        # snapshot the next instruction index for phase attribution
        nm = nc.get_next_instruction_name()
        PHASES.append((int(nm.split('-')[1]), label))

    xcm = nc.dram_tensor("xcm", (NSEQ, C, T), bf16, kind="ExternalInput")
    # folded TimeMix weights: [PR, 128, 2, C] each
    wk1 = nc.dram_tensor("wk1", (PR, 128, 2, C), f8, kind="ExternalInput")
    wk2 = nc.dram_tensor("wk2", (PR, 128, 2, C), f8, kind="ExternalInput")
    wv1 = nc.dram_tensor("wv1", (PR, 128, 2, C), f8, kind="ExternalInput")
    wv2 = nc.dram_tensor("wv2", (PR, 128, 2, C), f8, kind="ExternalInput")
    wr1 = nc.dram_tensor("wr1", (PR, 128, 2, C), f8, kind="ExternalInput")
    wr2 = nc.dram_tensor("wr2", (PR, 128, 2, C), f8, kind="ExternalInput")
    wo8 = nc.dram_tensor("wo8", (PR, 128, 2, C), f8, kind="ExternalInput")
    wck8 = nc.dram_tensor("wck8", (HB, 128, PR, 2, 128), f8, kind="ExternalInput")
    wcv8 = nc.dram_tensor("wcv8", (CB, 128, HPR, 2, 128), f8, kind="ExternalInput")
    wcr8 = nc.dram_tensor("wcr8", (CB, 128, PR, 2, 128), f8, kind="ExternalInput")
    vec6 = nc.dram_tensor("vec6", (6, C), f32, kind="ExternalInput")
    mixcm = nc.dram_tensor("mixcm", (NSEQ, C), f32, kind="ExternalInput")
    oct_ = nc.dram_tensor("oct", (NSEQ, C, T), f32, kind="ExternalOutput")
    dbg = {}
    if debug:
        dbg['h'] = nc.dram_tensor("dbg_h", (NCH, PR, 128, 2, TC + 1), f8,
                                  kind="ExternalOutput")
        for n in ('k', 'v', 'r'):
            dbg[n] = nc.dram_tensor(f"dbg_{n}", (NCH, CB, 128, TC), f32,
                                    kind="ExternalOutput")
        dbg['sry'] = nc.dram_tensor("dbg_sry", (NCH, PR, 128, 2, TC), f8,
                                    kind="ExternalOutput")
        dbg['x2'] = nc.dram_tensor("dbg_x2", (NCH, PR, 128, 2, TC), bf16,
                                   kind="ExternalOutput")
        dbg['mix'] = nc.dram_tensor("dbg_mix", (NCH, PR, 128, 2, TC), f8,
                                    kind="ExternalOutput")
        dbg['kk'] = nc.dram_tensor("dbg_kk", (NCH, HPR, 128, 2, TC), f8,
                                   kind="ExternalOutput")
        dbg['num'] = nc.dram_tensor("dbg_num", (NCH, CB, 128, TC), f32,
                                    kind="ExternalOutput")
        dbg['den'] = nc.dram_tensor("dbg_den", (NCH, CB, 128, TC), f32,
                                    kind="ExternalOutput")

    from contextlib import ExitStack
    with ExitStack() as ctx:
        tc = ctx.enter_context(tile.TileContext(nc))
        pc = ctx.enter_context(tc.tile_pool(name="const", bufs=1))
        pw = ctx.enter_context(tc.tile_pool(name="wres", bufs=1))
        pwck = ctx.enter_context(tc.tile_pool(name="wckst", bufs=6))
        pwcv = ctx.enter_context(tc.tile_pool(name="wcvst", bufs=2))
        px = ctx.enter_context(tc.tile_pool(name="x", bufs=8))
        psq = ctx.enter_context(tc.tile_pool(name="sq", bufs=2))
        ph = ctx.enter_context(tc.tile_pool(name="h", bufs=8))
        pd = ctx.enter_context(tc.tile_pool(name="d", bufs=3))
        pstat = ctx.enter_context(tc.tile_pool(name="stat", bufs=2))
        pbc = ctx.enter_context(tc.tile_pool(name="bcc", bufs=4))
        pg = ctx.enter_context(tc.tile_pool(name="gen", bufs=12))
        pf32 = ctx.enter_context(tc.tile_pool(name="f32s", bufs=4))
        pga = ctx.enter_context(tc.tile_pool(name="genA", bufs=4))
        psry = ctx.enter_context(tc.tile_pool(name="sry", bufs=4))
        px2 = ctx.enter_context(tc.tile_pool(name="x2", bufs=6))
        ph2 = ctx.enter_context(tc.tile_pool(name="h2", bufs=8, ))
        pmix = ctx.enter_context(tc.tile_pool(name="mix", bufs=8))
        prelu = ctx.enter_context(tc.tile_pool(name="relu", bufs=3))
        pkk = ctx.enter_context(tc.tile_pool(name="kk", bufs=18))
        pout = ctx.enter_context(tc.tile_pool(name="out", bufs=2))
        psmm = ctx.enter_context(tc.tile_pool(name="psmm", bufs=6, space="PSUM"))
        psst = ctx.enter_context(tc.tile_pool(name="pss", bufs=2, space="PSUM"))

        # ---- constants ----
        invCb = pc.tile([128, 1], bf16, tag="invCb")
        nc.vector.memset(invCb[:], 1.0 / C)
        eps_t = pc.tile([128, 1], f32, tag="eps")
        nc.vector.memset(eps_t[:], EPS)
        negone_f = pc.tile([1, 1], f32, tag="negonef")
        nc.vector.memset(negone_f[:], -1.0)
        negone = pc.tile([1, 1], f32r, tag="negone")
        nc.scalar.copy(negone[:], negone_f[:])

        def colload(src_ap, ncol, dtype=f32, tag=None):
            t = pc.tile([128, ncol], dtype, tag=tag)
            nc.sync.dma_start(t[:], src_ap)
            return t

        lam_c = colload(vec6[0].rearrange("(j p) -> p j", p=128), CB, tag="lam")
        eu_c = colload(vec6[1].rearrange("(j p) -> p j", p=128), CB, tag="eu")
        g1_c = colload(vec6[2].rearrange("(j p) -> p j", p=128), CB, tag="g1")
        b1_c = colload(vec6[3].rearrange("(j p) -> p j", p=128), CB, tag="b1")
        g2_c = colload(vec6[4].rearrange("(j p) -> p j", p=128), CB, tag="g2")
        b2_c = colload(vec6[5].rearrange("(j p) -> p j", p=128), CB, tag="b2")
        cmk_c = colload(mixcm.rearrange("s (j p) -> p (s j)", p=128),
                        NSEQ * CB, tag="cmk")

        # carries (chunk -> chunk)
        carryH = pc.tile([128, NSEQ * CB], f8, tag="carryH")     # h pairs
        carryH2 = pc.tile([128, NSEQ * CB], bf16, tag="carryH2")  # h2
        carryA = pc.tile([128, NSEQ * CB], bf16, tag="carryA")
        carryB = pc.tile([128, NSEQ * CB], bf16, tag="carryB")

        # ---- resident weights ----
        def wload(src, tag):
            tiles = []
            for j in range(PR):
                t = pw.tile([128, 2, C], f8, tag=f"{tag}{j}")
                nc.sync.dma_start(t[:], src[j])
                tiles.append(t)
            return tiles

        wk1_sb = wload(wk1, "wk1")
        wk2_sb = wload(wk2, "wk2")
        wv1_sb = wload(wv1, "wv1")
        wv2_sb = wload(wv2, "wv2")
        wr1_sb = wload(wr1, "wr1")
        wr2_sb = wload(wr2, "wr2")
        wo_sb = wload(wo8, "wo")
        wcr_sb = []
        for cb in range(CB):
            t = pw.tile([128, PR, 2, 128], f8, tag=f"wcr{cb}")
            nc.sync.dma_start(t[:], wcr8[cb])
            wcr_sb.append(t)

        def ln_begin():
            s1 = psst.tile([1, TC], f32, tag="ss", name="s1")
            s2 = psst.tile([1, TC], f32, tag="ss", name="s2")
            return s1, s2

        def ln_feed(st, pair, j):
            """accumulate pair j's s1/s2 contributions (PE + one DVE sq)"""
            s1, s2 = st
            for i in range(2):
                nc.tensor.matmul(s1[:], invCb[:], pair[:, i, :],
                                 start=(j == 0 and i == 0),
                                 stop=(j == PR - 1 and i == 1))
                sq = psq.tile([128, TC], bf16, tag="sq")
                nc.vector.tensor_tensor(sq[:], pair[:, i, :],
                                        pair[:, i, :], AL.mult)
                nc.tensor.matmul(s2[:], invCb[:], sq[:],
                                 start=(j == 0 and i == 0), stop=False)

        def ln_finish(st):
            """var = s2 - mu^2 via an extra PSUM accumulation matmul so the
            stats chain stays PE<->Act only; returns broadcast (mu_sb, rs_sb)."""
            s1, s2 = st
            mu2 = pstat.tile([1, TC], f32r, tag="st")
            nc.scalar.activation(mu2[:], s1[:], AF.Square)
            mub = pstat.tile([1, TC], bf16, tag="st")
            nc.scalar.activation(mub[:], s1[:], AF.Copy)
            nc.tensor.matmul(s2[:], negone[:], mu2[:], start=False, stop=True)
            lnv = pstat.tile([1, TC], f32, tag="st")
            nc.scalar.activation(lnv[:], s2[:], AF.Ln, bias=eps_t[0:1, 0:1])
            rs = pstat.tile([1, TC], bf16, tag="st")
            nc.scalar.activation(rs[:], lnv[:], AF.Exp, scale=-0.5)
            mu_sb = pbc.tile([128, TC], bf16, tag="mu_sb")
            nc.gpsimd.partition_broadcast(mu_sb[:], mub[:], 128)
            rs_sb = pbc.tile([128, TC], bf16, tag="rs_sb")
            nc.gpsimd.partition_broadcast(rs_sb[:], rs[:], 128)
            return mu_sb, rs_sb

        def ln_stats(src_pairs, g_c, b_c, use_gb):
            st = ln_begin()
            for j in range(PR):
                ln_feed(st, src_pairs[j], j)
            return ln_finish(st)

        def tm1_stats(s, ch):
            xts = []
            xsrc = xcm[s].rearrange("(j i p) t -> j p i t", i=2, p=128)
            for j in range(PR):
                xt = px.tile([128, 2, TC], bf16, tag="x")
                nc.sync.dma_start(xt[:], xsrc[j, :, :, ch * TC:(ch + 1) * TC])
                xts.append(xt)
            mu_sb, rs_sb = ln_stats(xts, g1_c, b1_c, use_gb1)
            return xts, mu_sb, rs_sb

        def tm1_apply(s, ch, st):
            xts, mu_sb, rs_sb = st
            hts = []
            for j in range(PR):
                ht = ph.tile([128, 2, TC + 1], f8, tag="h")
                for i in range(2):
                    cb = 2 * j + i
                    t = pd.tile([128, TC], bf16, tag="d")
                    nc.vector.tensor_tensor(t[:], xts[j][:, i, :], mu_sb[:],
                                            AL.subtract)
                    if use_gb1:
                        tg = pd.tile([128, TC], bf16, tag="d")
                        nc.vector.tensor_tensor(tg[:], t[:], rs_sb[:], AL.mult)
                        nc.vector.tensor_scalar(
                            ht[:, i, 1:TC + 1], tg[:], g_col(g1_c, cb),
                            b_col(b1_c, cb), AL.mult, AL.add)
                    else:
                        nc.gpsimd.tensor_tensor(ht[:, i, 1:TC + 1], t[:],
                                                rs_sb[:], AL.mult)
                idx = s * CB + 2 * j
                if ch == 0:
                    nc.vector.memset(ht[:, :, 0:1], 0.0)
                else:
                    nc.vector.tensor_copy(ht[:, :, 0:1], carryH[:, idx:idx + 2])
                if ch < NCH - 1:
                    nc.vector.tensor_copy(carryH[:, idx:idx + 2],
                                          ht[:, :, TC:TC + 1])
                if debug and s == 0:
                    nc.sync.dma_start(dbg['h'][ch, j], ht[:])
                hts.append(ht)
            return hts

        def g_col(gc, cb):
            return gc[:, cb:cb + 1]

        def mm8(psum, w1_sb, w2_sb, hts, db):
            """accumulate W1 @ h + W2 @ shift(h) into psum (8 DR matmuls)"""
            lo, hi = db * 128, (db + 1) * 128
            for j in range(PR):
                nc.tensor.matmul(psum[:], w1_sb[j][:, :, lo:hi],
                                 hts[j][:, :, 1:TC + 1],
                                 start=(j == 0), stop=False,
                                 perf_mode=PM.DoubleRow)
            for j in range(PR):
                nc.tensor.matmul(psum[:], w2_sb[j][:, :, lo:hi],
                                 hts[j][:, :, 0:TC],
                                 start=False, stop=(j == PR - 1),
                                 perf_mode=PM.DoubleRow)

        def tm2(s, ch, xts, hts):
            sry_ts = [psry.tile([128, 2, TC], f8, tag="sry", name=f"sry{_j}")
                      for _j in range(PR)]
            for db in range(CB):
                idx = s * CB + db
                kps = psmm.tile([128, TC], f32, tag="mm")
                mm8(kps, wk1_sb, wk2_sb, hts, db)
                vps = psmm.tile([128, TC], f32, tag="mm")
                mm8(vps, wv1_sb, wv2_sb, hts, db)
                rps = psmm.tile([128, TC], f32, tag="mm")
                mm8(rps, wr1_sb, wr2_sb, hts, db)
                if debug and s == 0:
                    for nme, ps in (('k', kps), ('v', vps), ('r', rps)):
                        dtmp = pd.tile([128, TC], f32, tag="d",
                                       name=f"dbg{nme}{db}")
                        nc.scalar.activation(dtmp[:], ps[:], AF.Copy,
                                             scale=1.0 / WS)
                        nc.sync.dma_start(dbg[nme][ch, db], dtmp[:])

                ek = pg.tile([128, TC], bf16, tag="gen")
                nc.scalar.activation(ek[:], kps[:], AF.Exp, scale=1.0 / WS)
                eku = pg.tile([128, TC], bf16, tag="gen")
                nc.scalar.activation(eku[:], kps[:], AF.Exp, scale=1.0 / WS,
                                     bias=eu_c[:, db:db + 1])
                vbf = pg.tile([128, TC], bf16, tag="gen")
                nc.scalar.activation(vbf[:], vps[:], AF.Copy, scale=1.0 / WS)
                enr = pg.tile([128, TC], bf16, tag="gen")
                nc.scalar.activation(enr[:], rps[:], AF.Exp, scale=-1.0 / WS)

                ekv = pg.tile([128, TC], bf16, tag="gen")
                nc.gpsimd.tensor_tensor(ekv[:], ek[:], vbf[:], AL.mult)
                ekuv = pg.tile([128, TC], bf16, tag="gen")
                nc.gpsimd.tensor_tensor(ekuv[:], eku[:], vbf[:], AL.mult)

                At = pga.tile([128, TC + 1], bf16, tag="genA")
                Bt = pga.tile([128, TC + 1], bf16, tag="genA")
                if ch == 0:
                    nc.vector.memset(At[:, 0:1], 0.0)
                    nc.vector.memset(Bt[:, 0:1], 0.0)
                else:
                    nc.vector.tensor_copy(At[:, 0:1], carryA[:, idx:idx + 1])
                    nc.vector.tensor_copy(Bt[:, 0:1], carryB[:, idx:idx + 1])
                lamb = lam_c[:, db:db + 1].broadcast_to((128, TC))
                nc.vector.tensor_tensor_scan(At[:, 1:TC + 1], lamb, ekv[:],
                                             At[:, 0:1], AL.mult, AL.add)
                nc.vector.tensor_tensor_scan(Bt[:, 1:TC + 1], lamb, ek[:],
                                             Bt[:, 0:1], AL.mult, AL.add)
                if ch < NCH - 1:
                    nc.vector.tensor_copy(carryA[:, idx:idx + 1],
                                          At[:, TC:TC + 1])
                    nc.vector.tensor_copy(carryB[:, idx:idx + 1],
                                          Bt[:, TC:TC + 1])

                num = pg.tile([128, TC], bf16, tag="gen")
                nc.vector.tensor_tensor(num[:], At[:, 0:TC], ekuv[:], AL.add)
                den = pg.tile([128, TC], bf16, tag="gen")
                nc.vector.tensor_tensor(den[:], Bt[:, 0:TC], eku[:], AL.add)
                enr1 = pg.tile([128, TC], bf16, tag="gen")
                nc.vector.tensor_scalar_add(enr1[:], enr[:], 1.0)
                den2 = pf32.tile([128, TC], f32, tag="f32s")
                nc.vector.tensor_tensor(den2[:], den[:], enr1[:], AL.mult)
                rec = pf32.tile([128, TC], f32, tag="f32s")
                nc.vector.reciprocal_approx_fast(rec[:], den2[:])
                nc.vector.tensor_tensor(sry_ts[db // 2][:, db % 2, :],
                                        num[:], rec[:], AL.mult)
                if debug and s == 0:
                    for nme, src in (('num', num), ('den', den)):
                        dtmp2 = pd.tile([128, TC], f32, tag="d",
                                        name=f"dbg{nme}{db}")
                        nc.vector.tensor_copy(dtmp2[:], src[:])
                        nc.sync.dma_start(dbg[nme][ch, db], dtmp2[:])

            x2_ts = []
            for j in range(PR):
                x2t = px2.tile([128, 2, TC], bf16, tag="x2")
                for i in range(2):
                    cb = 2 * j + i
                    xps = psmm.tile([128, TC], f32, tag="mm")
                    for jj in range(PR):
                        nc.tensor.matmul(
                            xps[:], wo_sb[jj][:, :, cb * 128:(cb + 1) * 128],
                            sry_ts[jj][:], start=(jj == 0), stop=(jj == PR - 1),
                            perf_mode=PM.DoubleRow)
                    xevac = pd.tile([128, TC], bf16, tag="d")
                    nc.scalar.activation(xevac[:], xps[:], AF.Copy,
                                         scale=1.0 / WS)
                    nc.vector.tensor_tensor(x2t[:, i, :], xevac[:],
                                            xts[j][:, i, :], AL.add)
                if debug and s == 0:
                    nc.sync.dma_start(dbg['sry'][ch, j], sry_ts[j][:])
                    nc.sync.dma_start(dbg['x2'][ch, j], x2t[:])
                x2_ts.append(x2t)
            return x2_ts

        def cm1(s, ch, x2_ts):
            mu_sb, rs_sb = ln_stats(x2_ts, g2_c, b2_c, use_gb2)
            h2ts = []
            for cb in range(CB):
                j, i = cb // 2, cb % 2
                h2t = ph2.tile([128, TC + 1], bf16, tag="h2")
                t = pd.tile([128, TC], bf16, tag="d")
                nc.vector.tensor_tensor(t[:], x2_ts[j][:, i, :], mu_sb[:],
                                        AL.subtract)
                nc.gpsimd.tensor_tensor(h2t[:, 1:TC + 1], t[:], rs_sb[:],
                                        AL.mult)
                if use_gb2:
                    nc.vector.tensor_scalar(h2t[:, 1:TC + 1], h2t[:, 1:TC + 1],
                                            g_col(g2_c, cb), g_col(b2_c, cb),
                                            AL.mult, AL.add)
                idx = s * CB + cb
                if ch == 0:
                    nc.vector.memset(h2t[:, 0:1], 0.0)
                else:
                    nc.vector.tensor_copy(h2t[:, 0:1], carryH2[:, idx:idx + 1])
                if ch < NCH - 1:
                    nc.vector.tensor_copy(carryH2[:, idx:idx + 1],
                                          h2t[:, TC:TC + 1])
                h2ts.append(h2t)
            mix_ts = [pmix.tile([128, 2, TC], f8, tag="mix", name=f"mix{_j}")
                      for _j in range(PR)]
            for cb in range(CB):
                idx = s * CB + cb
                dt = pd.tile([128, TC], bf16, tag="d")
                nc.vector.tensor_tensor(dt[:], h2ts[cb][:, 1:TC + 1],
                                        h2ts[cb][:, 0:TC], AL.subtract)
                nc.vector.scalar_tensor_tensor(
                    mix_ts[cb // 2][:, cb % 2, :], dt[:],
                    cmk_c[:, idx:idx + 1], h2ts[cb][:, 0:TC], AL.mult, AL.add)
            kk_ts = [pkk.tile([128, 2, TC], f8, tag="kk", name=f"kk{_j}")
                     for _j in range(HPR)]
            for hb in range(HB):
                wckt = pwck.tile([128, PR, 2, 128], f8, tag="wck")
                nc.sync.dma_start(wckt[:], wck8[hb])
                ckps = psmm.tile([128, TC], f32, tag="mm")
                for j in range(PR):
                    nc.tensor.matmul(ckps[:], wckt[:, j], mix_ts[j][:],
                                     start=(j == 0), stop=(j == PR - 1),
                                     perf_mode=PM.DoubleRow)
                relu = prelu.tile([128, TC], bf16, tag="relu")
                nc.scalar.activation(relu[:], ckps[:], AF.Relu, scale=1.0 / WS)
                if hb % 2 == 0:
                    nc.gpsimd.tensor_tensor(kk_ts[hb // 2][:, hb % 2, :],
                                            relu[:], relu[:], AL.mult)
                else:
                    nc.vector.tensor_tensor(kk_ts[hb // 2][:, hb % 2, :],
                                            relu[:], relu[:], AL.mult)
            if debug and s == 0:
                for j in range(PR):
                    nc.sync.dma_start(dbg['mix'][ch, j], mix_ts[j][:])
                for j in range(HPR):
                    nc.sync.dma_start(dbg['kk'][ch, j], kk_ts[j][:])
            return x2_ts, mix_ts, kk_ts

        def cm2(s, ch, st3):
            x2_ts, mix_ts, kk_ts = st3
            for cb in range(CB):
                crps = psmm.tile([128, TC], f32, tag="mm")
                for j in range(PR):
                    nc.tensor.matmul(crps[:], wcr_sb[cb][:, j], mix_ts[j][:],
                                     start=(j == 0), stop=(j == PR - 1),
                                     perf_mode=PM.DoubleRow)
                enz = pg.tile([128, TC], bf16, tag="gen")
                nc.scalar.activation(enz[:], crps[:], AF.Exp, scale=-1.0 / WS)
                enz1 = pf32.tile([128, TC], f32, tag="f32s")
                nc.vector.tensor_scalar(enz1[:], enz[:], WS, WS,
                                        AL.mult, AL.add)
                rec = pf32.tile([128, TC], f32, tag="f32s")
                nc.vector.reciprocal_approx_fast(rec[:], enz1[:])
                wcvt = pwcv.tile([128, HPR, 2, 128], f8, tag="wcv")
                nc.sync.dma_start(wcvt[:], wcv8[cb])
                kvps = psmm.tile([128, TC], f32, tag="mm")
                for j in range(HPR):
                    nc.tensor.matmul(kvps[:], wcvt[:, j], kk_ts[j][:],
                                     start=(j == 0), stop=(j == HPR - 1),
                                     perf_mode=PM.DoubleRow)
                t1 = pd.tile([128, TC], bf16, tag="d")
                nc.vector.tensor_tensor(t1[:], kvps[:], rec[:], AL.mult)
                outt = pout.tile([128, TC], f32, tag="out")
                nc.vector.tensor_tensor(outt[:], x2_ts[cb // 2][:, cb % 2, :],
                                        t1[:], AL.add)
                nc.sync.dma_start(
                    oct_[s, cb * 128:(cb + 1) * 128, ch * TC:(ch + 1) * TC],
                    outt[:])

        # software-pipelined emission as in the baseline:
        # tm1(u) -> cm2(prev) -> tm2(u) -> cm1(u)
        # ch-major: consecutive units belong to different sequences, so
        # their chains are independent and overlap on every engine.
        units = [(s, ch) for ch in range(NCH) for s in range(NSEQ)]
        cm1_st = {}
        prev = None
        for u in units:
            mark(f"tm1{u}")
            st_u = tm1_stats(*u)
            hts_u = tm1_apply(*u, st_u)
            if prev is not None:
                mark(f"cm2{prev}")
                cm2(*prev, cm1_st.pop(prev))
            mark(f"tm2{u}")
            x2_ts = tm2(*u, st_u[0], hts_u)
            mark(f"cm1{u}")
            cm1_st[u] = cm1(*u, x2_ts)
            prev = u
        mark(f"cm2{prev}")
        cm2(*prev, cm1_st.pop(prev))
        mark("end")

    nc.compile()
    return nc


def _prep_weights(inputs):
    """Host-side fp8 weight prep. Returns dict of arrays shared by all cores
    plus per-p folded TimeMix weights."""
    bf = ml_dtypes.bfloat16
    f8 = ml_dtypes.float8_e4m3

    def q8(a):
        return np.clip(a * WS, -240, 240).astype(f8)

    def fold_pair(W, m):
        # lhsT[c, d] = W[d, c] * m[c]; layout [j, p, i, d]
        WT = np.asarray(W, np.float32).T * m[:, None]
        return np.ascontiguousarray(
            WT.reshape(PR, 2, 128, C).transpose(0, 2, 1, 3))

    def plain_pair(W):
        WT = np.ascontiguousarray(np.asarray(W, np.float32).T)
        return np.ascontiguousarray(
            WT.reshape(PR, 2, 128, C).transpose(0, 2, 1, 3))

    out = {}
    mk = np.asarray(inputs['att_mix_k'], np.float32).reshape(P, C)
    mv = np.asarray(inputs['att_mix_v'], np.float32).reshape(P, C)
    mr = np.asarray(inputs['att_mix_r'], np.float32).reshape(P, C)
    for p in range(P):
        out[p] = {
            'wk1': q8(fold_pair(inputs['Wk'], mk[p])),
            'wk2': q8(fold_pair(inputs['Wk'], 1 - mk[p])),
            'wv1': q8(fold_pair(inputs['Wv'], mv[p])),
            'wv2': q8(fold_pair(inputs['Wv'], 1 - mv[p])),
            'wr1': q8(fold_pair(inputs['Wr'], mr[p])),
            'wr2': q8(fold_pair(inputs['Wr'], 1 - mr[p])),
        }
    shared = {'wo8': q8(plain_pair(inputs['Wo']))}
    # wck8[hb, p, j, i, dd] = Wck.T[(2j+i)*128+p, hb*128+dd] * WS
    WckT = np.asarray(inputs['Wck'], np.float32).T
    shared['wck8'] = q8(np.ascontiguousarray(
        WckT.reshape(PR, 2, 128, HB, 128).transpose(3, 2, 0, 1, 4)))
    WcvT = np.asarray(inputs['Wcv'], np.float32).T
    shared['wcv8'] = q8(np.ascontiguousarray(
        WcvT.reshape(HPR, 2, 128, CB, 128).transpose(3, 2, 0, 1, 4)))
    WcrT = np.asarray(inputs['Wcr'], np.float32).T
    shared['wcr8'] = q8(np.ascontiguousarray(
        WcrT.reshape(PR, 2, 128, CB, 128).transpose(3, 2, 0, 1, 4)))
    return out, shared


def kernel(**inputs):
    from concourse.bass_utils import run_bass_kernel_spmd

    x = np.asarray(inputs['x'], dtype=np.float32)
    g1 = np.asarray(inputs['ln1_g'], np.float32)
    b1 = np.asarray(inputs['ln1_b'], np.float32)
    g2 = np.asarray(inputs['ln2_g'], np.float32)
    b2 = np.asarray(inputs['ln2_b'], np.float32)
    use_gb1 = not (np.all(g1 == 1.0) and np.all(b1 == 0.0))
    use_gb2 = not (np.all(g2 == 1.0) and np.all(b2 == 0.0))
    cmk = np.asarray(inputs['cm_mix_k'], np.float32).reshape(P, C)
    cmr = np.asarray(inputs['cm_mix_r'], np.float32).reshape(P, C)
    assert np.array_equal(cmk, cmr), "kernel2 assumes cm_mix_k == cm_mix_r"

    debug = os.environ.get('RWKV_DEBUG', '0') == '1'
    key = (use_gb1, use_gb2, debug)
    if key not in _CACHE:
        _CACHE[key] = _build(use_gb1, use_gb2, debug)
    nc = _CACHE[key]

    bf = ml_dtypes.bfloat16
    lam = np.exp(-np.exp(np.asarray(inputs['time_decay'], np.float32)))
    # row 1 is raw u = time_first: it enters as the exp() bias on the device
    u = np.asarray(inputs['time_first'], np.float32)
    vec6 = np.stack([lam.astype(np.float32), u,
                     g1, b1, g2, b2]).astype(np.float32)

    perp, shared = _prep_weights(inputs)

    xf = x.reshape(P * B, T, C)
    in_maps = []
    for core in range(NCORES):
        seqs = [2 * core, 2 * core + 1]
        p = seqs[0] // B
        assert seqs[1] // B == p
        xcm = np.ascontiguousarray(xf[seqs].transpose(0, 2, 1)).astype(bf)
        in_maps.append({
            'xcm': xcm, 'vec6': vec6,
            'mixcm': np.stack([cmk[p], cmk[p]]).astype(np.float32),
            **perp[p], **shared,
        })

    trace = os.environ.get('RWKV_TRACE', '0') == '1'
    res = run_bass_kernel_spmd(nc, in_maps, list(range(NCORES)), trace=trace)
    global LAST_RUN_INFO
    LAST_RUN_INFO = res

    out = np.empty((P * B, T, C), np.float32)
    for core in range(NCORES):
        oc = res.results[core]['oct']
        out[2 * core] = oc[0].T
        out[2 * core + 1] = oc[1].T
    return out.reshape(P, B, T, C)


LAST_RUN_INFO = None


# revision 3
# speedup vs baseline: 1.0383x; 1.0383x over previous
# RWKV-v4 block (TimeMix WKV + ChannelMix) on 8 Trainium2 NeuronCores — v2.
#
# Sharding: data-parallel over the 16 (p, b) sequences -> 2 per core (both
# sequences of a core share the same p, so mix/fold constants are per-core).
#
# v2 strategy vs the bf16 baseline (924us cost-model):
# - All seven matmuls run as fp8e4m3 DoubleRow (2 k-tiles interleaved,
#   K=256/instr at 0.5 cyc/row): 4x fewer PE cycles than bf16.
# - TimeMix token-shift mixes are folded into the weights:
#   k = (Wk*diag(mk)) @ h + (Wk*diag(1-mk)) @ shift(h); h itself is written
#   directly in fp8 pair tiles [128, 2, TC+1] (col 0 = carry), so no mix
#   tiles and no shift ops exist on the vector engines for the k/v/r paths.
# - Weights are host-prescaled by WS=64 into the e4m3 normal range; every
#   inverse scale folds into an Activation scale= or the sigmoid chain.
# - WKV pipeline in bf16 (2x DVE mode on tensor_tensor), scans keep f32
#   lambda, reciprocals in f32 (hw requirement).
# - ChannelMix: cm_mix_k == cm_mix_r (runtime-checked) -> one shared mix
#   tile in fp8; Wck/Wcv are streamed per-unit in hb-/cb-blocked fp8 pair
#   layouts; relu on Act, squares split Pool/DVE -> fp8 (splitting the
#   32-op serial run across engines shortens the per-unit critical path).
import os
import numpy as np
import ml_dtypes

P, B, T, C = 2, 8, 1024, 1024
H = 4 * C
NCORES = 8
NSEQ = 2
TC = 512
NCH = T // TC      # 2
CB = C // 128      # 8
PR = CB // 2       # 4 channel-block pairs
HB = H // 128      # 32
HPR = HB // 2      # 16
EPS = 1e-5
WS = 64.0          # fp8 weight prescale

_CACHE = {}
PHASES = []


def _build(use_gb1, use_gb2, debug=False):
    import concourse.bass as bass
    import concourse.tile as tile
    from concourse import bacc, mybir

    f32 = mybir.dt.float32
    f32r = mybir.dt.float32r
    bf16 = mybir.dt.bfloat16
    f8 = mybir.dt.float8e4
    AL = mybir.AluOpType
    AF = mybir.ActivationFunctionType
    PM = mybir.MatmulPerfMode

    nc = bacc.Bacc()
    global PHASES
    PHASES = []

    def mark(label):
        # snapshot the next instruction index for phase attribution
        nm = nc.get_next_instruction_name()
        PHASES.append((int(nm.split('-')[1]), label))

    xcm = nc.dram_tensor("xcm", (NSEQ, C, T), bf16, kind="ExternalInput")
    # folded TimeMix weights: [PR, 128, 2, C] each
    wk1 = nc.dram_tensor("wk1", (PR, 128, 2, C), f8, kind="ExternalInput")
    wk2 = nc.dram_tensor("wk2", (PR, 128, 2, C), f8, kind="ExternalInput")
    wv1 = nc.dram_tensor("wv1", (PR, 128, 2, C), f8, kind="ExternalInput")
    wv2 = nc.dram_tensor("wv2", (PR, 128, 2, C), f8, kind="ExternalInput")
    wr1 = nc.dram_tensor("wr1", (PR, 128, 2, C), f8, kind="ExternalInput")
    wr2 = nc.dram_tensor("wr2", (PR, 128, 2, C), f8, kind="ExternalInput")
    wo8 = nc.dram_tensor("wo8", (PR, 128, 2, C), f8, kind="ExternalInput")
    wck8 = nc.dram_tensor("wck8", (HB, 128, PR, 2, 128), f8, kind="ExternalInput")
    wcv8 = nc.dram_tensor("wcv8", (CB, 128, HPR, 2, 128), f8, kind="ExternalInput")
    wcr8 = nc.dram_tensor("wcr8", (CB, 128, PR, 2, 128), f8, kind="ExternalInput")
    vec6 = nc.dram_tensor("vec6", (6, C), f32, kind="ExternalInput")
    mixcm = nc.dram_tensor("mixcm", (NSEQ, C), f32, kind="ExternalInput")
    oct_ = nc.dram_tensor("oct", (NSEQ, C, T), f32, kind="ExternalOutput")
    dbg = {}
    if debug:
        dbg['h'] = nc.dram_tensor("dbg_h", (NCH, PR, 128, 2, TC + 1), f8,
                                  kind="ExternalOutput")
        for n in ('k', 'v', 'r'):
            dbg[n] = nc.dram_tensor(f"dbg_{n}", (NCH, CB, 128, TC), f32,
                                    kind="ExternalOutput")
        dbg['sry'] = nc.dram_tensor("dbg_sry", (NCH, PR, 128, 2, TC), f8,
                                    kind="ExternalOutput")
        dbg['x2'] = nc.dram_tensor("dbg_x2", (NCH, PR, 128, 2, TC), bf16,
                                   kind="ExternalOutput")
        dbg['mix'] = nc.dram_tensor("dbg_mix", (NCH, PR, 128, 2, TC), f8,
                                    kind="ExternalOutput")
        dbg['kk'] = nc.dram_tensor("dbg_kk", (NCH, HPR, 128, 2, TC), f8,
                                   kind="ExternalOutput")
        dbg['num'] = nc.dram_tensor("dbg_num", (NCH, CB, 128, TC), f32,
                                    kind="ExternalOutput")
        dbg['den'] = nc.dram_tensor("dbg_den", (NCH, CB, 128, TC), f32,
                                    kind="ExternalOutput")

    from contextlib import ExitStack
    with ExitStack() as ctx:
        tc = ctx.enter_context(tile.TileContext(nc))
        pc = ctx.enter_context(tc.tile_pool(name="const", bufs=1))
        pw = ctx.enter_context(tc.tile_pool(name="wres", bufs=1))
        pwck = ctx.enter_context(tc.tile_pool(name="wckst", bufs=6))
        pwcv = ctx.enter_context(tc.tile_pool(name="wcvst", bufs=2))
        px = ctx.enter_context(tc.tile_pool(name="x", bufs=8))
        psq = ctx.enter_context(tc.tile_pool(name="sq", bufs=2))
        ph = ctx.enter_context(tc.tile_pool(name="h", bufs=8))
        pd = ctx.enter_context(tc.tile_pool(name="d", bufs=3))
        pstat = ctx.enter_context(tc.tile_pool(name="stat", bufs=2))
        pbc = ctx.enter_context(tc.tile_pool(name="bcc", bufs=4))
        pg = ctx.enter_context(tc.tile_pool(name="gen", bufs=12))
        pf32 = ctx.enter_context(tc.tile_pool(name="f32s", bufs=4))
        pga = ctx.enter_context(tc.tile_pool(name="genA", bufs=4))
        psry = ctx.enter_context(tc.tile_pool(name="sry", bufs=4))
        px2 = ctx.enter_context(tc.tile_pool(name="x2", bufs=6))
        ph2 = ctx.enter_context(tc.tile_pool(name="h2", bufs=8, ))
        pmix = ctx.enter_context(tc.tile_pool(name="mix", bufs=8))
        prelu = ctx.enter_context(tc.tile_pool(name="relu", bufs=3))
        pkk = ctx.enter_context(tc.tile_pool(name="kk", bufs=18))
        pout = ctx.enter_context(tc.tile_pool(name="out", bufs=2))
        psmm = ctx.enter_context(tc.tile_pool(name="psmm", bufs=6, space="PSUM"))
        psst = ctx.enter_context(tc.tile_pool(name="pss", bufs=2, space="PSUM"))

        # ---- constants ----
        invCb = pc.tile([128, 1], bf16, tag="invCb")
        nc.vector.memset(invCb[:], 1.0 / C)
        eps_t = pc.tile([128, 1], f32, tag="eps")
        nc.vector.memset(eps_t[:], EPS)
        negone_f = pc.tile([1, 1], f32, tag="negonef")
        nc.vector.memset(negone_f[:], -1.0)
        negone = pc.tile([1, 1], f32r, tag="negone")
        nc.scalar.copy(negone[:], negone_f[:])

        def colload(src_ap, ncol, dtype=f32, tag=None):
            t = pc.tile([128, ncol], dtype, tag=tag)
            nc.sync.dma_start(t[:], src_ap)
            return t

        lam_c = colload(vec6[0].rearrange("(j p) -> p j", p=128), CB, tag="lam")
        eu_c = colload(vec6[1].rearrange("(j p) -> p j", p=128), CB, tag="eu")
        g1_c = colload(vec6[2].rearrange("(j p) -> p j", p=128), CB, tag="g1")
        b1_c = colload(vec6[3].rearrange("(j p) -> p j", p=128), CB, tag="b1")
        g2_c = colload(vec6[4].rearrange("(j p) -> p j", p=128), CB, tag="g2")
        b2_c = colload(vec6[5].rearrange("(j p) -> p j", p=128), CB, tag="b2")
        cmk_c = colload(mixcm.rearrange("s (j p) -> p (s j)", p=128),
                        NSEQ * CB, tag="cmk")

        # carries (chunk -> chunk)
        carryH = pc.tile([128, NSEQ * CB], f8, tag="carryH")     # h pairs
        carryH2 = pc.tile([128, NSEQ * CB], bf16, tag="carryH2")  # h2
        carryA = pc.tile([128, NSEQ * CB], bf16, tag="carryA")
        carryB = pc.tile([128, NSEQ * CB], bf16, tag="carryB")

        # ---- resident weights ----
        def wload(src, tag):
            tiles = []
            for j in range(PR):
                t = pw.tile([128, 2, C], f8, tag=f"{tag}{j}")
                nc.sync.dma_start(t[:], src[j])
                tiles.append(t)
            return tiles

        wk1_sb = wload(wk1, "wk1")
        wk2_sb = wload(wk2, "wk2")
        wv1_sb = wload(wv1, "wv1")
        wv2_sb = wload(wv2, "wv2")
        wr1_sb = wload(wr1, "wr1")
        wr2_sb = wload(wr2, "wr2")
        wo_sb = wload(wo8, "wo")
        wcr_sb = []
        for cb in range(CB):
            t = pw.tile([128, PR, 2, 128], f8, tag=f"wcr{cb}")
            nc.sync.dma_start(t[:], wcr8[cb])
            wcr_sb.append(t)

        def ln_begin():
            s1 = psst.tile([1, TC], f32, tag="ss", name="s1")
            s2 = psst.tile([1, TC], f32, tag="ss", name="s2")
            return s1, s2

        def ln_feed(st, pair, j):
            """accumulate pair j's s1/s2 contributions (PE + one DVE sq)"""
            s1, s2 = st
            for i in range(2):
                nc.tensor.matmul(s1[:], invCb[:], pair[:, i, :],
                                 start=(j == 0 and i == 0),
                                 stop=(j == PR - 1 and i == 1))
                sq = psq.tile([128, TC], bf16, tag="sq")
                nc.vector.tensor_tensor(sq[:], pair[:, i, :],
                                        pair[:, i, :], AL.mult)
                nc.tensor.matmul(s2[:], invCb[:], sq[:],
                                 start=(j == 0 and i == 0), stop=False)

        def ln_finish(st):
            """var = s2 - mu^2 via an extra PSUM accumulation matmul so the
            stats chain stays PE<->Act only; returns broadcast (mu_sb, rs_sb)."""
            s1, s2 = st
            mu2 = pstat.tile([1, TC], f32r, tag="st")
            nc.scalar.activation(mu2[:], s1[:], AF.Square)
            mub = pstat.tile([1, TC], bf16, tag="st")
            nc.scalar.activation(mub[:], s1[:], AF.Copy)
            nc.tensor.matmul(s2[:], negone[:], mu2[:], start=False, stop=True)
            lnv = pstat.tile([1, TC], f32, tag="st")
            nc.scalar.activation(lnv[:], s2[:], AF.Ln, bias=eps_t[0:1, 0:1])
            rs = pstat.tile([1, TC], bf16, tag="st")
            nc.scalar.activation(rs[:], lnv[:], AF.Exp, scale=-0.5)
            mu_sb = pbc.tile([128, TC], bf16, tag="mu_sb")
            nc.gpsimd.partition_broadcast(mu_sb[:], mub[:], 128)
            rs_sb = pbc.tile([128, TC], bf16, tag="rs_sb")
            nc.gpsimd.partition_broadcast(rs_sb[:], rs[:], 128)
            return mu_sb, rs_sb

        def ln_stats(src_pairs, g_c, b_c, use_gb):
            st = ln_begin()
            for j in range(PR):
                ln_feed(st, src_pairs[j], j)
            return ln_finish(st)

        def tm1_stats(s, ch):
            xts = []
            xsrc = xcm[s].rearrange("(j i p) t -> j p i t", i=2, p=128)
            for j in range(PR):
                xt = px.tile([128, 2, TC], bf16, tag="x")
                nc.sync.dma_start(xt[:], xsrc[j, :, :, ch * TC:(ch + 1) * TC])
                xts.append(xt)
            mu_sb, rs_sb = ln_stats(xts, g1_c, b1_c, use_gb1)
            return xts, mu_sb, rs_sb

        def tm1_apply(s, ch, st):
            xts, mu_sb, rs_sb = st
            hts = []
            for j in range(PR):
                ht = ph.tile([128, 2, TC + 1], f8, tag="h")
                for i in range(2):
                    cb = 2 * j + i
                    t = pd.tile([128, TC], bf16, tag="d")
                    nc.vector.tensor_tensor(t[:], xts[j][:, i, :], mu_sb[:],
                                            AL.subtract)
                    if use_gb1:
                        tg = pd.tile([128, TC], bf16, tag="d")
                        nc.vector.tensor_tensor(tg[:], t[:], rs_sb[:], AL.mult)
                        nc.vector.tensor_scalar(
                            ht[:, i, 1:TC + 1], tg[:], g_col(g1_c, cb),
                            b_col(b1_c, cb), AL.mult, AL.add)
                    else:
                        nc.gpsimd.tensor_tensor(ht[:, i, 1:TC + 1], t[:],
                                                rs_sb[:], AL.mult)
                idx = s * CB + 2 * j
                if ch == 0:
                    nc.vector.memset(ht[:, :, 0:1], 0.0)
                else:
                    nc.vector.tensor_copy(ht[:, :, 0:1], carryH[:, idx:idx + 2])
                if ch < NCH - 1:
                    nc.vector.tensor_copy(carryH[:, idx:idx + 2],
                                          ht[:, :, TC:TC + 1])
                if debug and s == 0:
                    nc.sync.dma_start(dbg['h'][ch, j], ht[:])
                hts.append(ht)
            return hts

        def g_col(gc, cb):
            return gc[:, cb:cb + 1]

        def mm8(psum, w1_sb, w2_sb, hts, db):
            """accumulate W1 @ h + W2 @ shift(h) into psum (8 DR matmuls)"""
            lo, hi = db * 128, (db + 1) * 128
            for j in range(PR):
                nc.tensor.matmul(psum[:], w1_sb[j][:, :, lo:hi],
                                 hts[j][:, :, 1:TC + 1],
                                 start=(j == 0), stop=False,
                                 perf_mode=PM.DoubleRow)
            for j in range(PR):
                nc.tensor.matmul(psum[:], w2_sb[j][:, :, lo:hi],
                                 hts[j][:, :, 0:TC],
                                 start=False, stop=(j == PR - 1),
                                 perf_mode=PM.DoubleRow)

        def tm2(s, ch, xts, hts):
            sry_ts = [psry.tile([128, 2, TC], f8, tag="sry", name=f"sry{_j}")
                      for _j in range(PR)]
            for db in range(CB):
                idx = s * CB + db
                kps = psmm.tile([128, TC], f32, tag="mm")
                mm8(kps, wk1_sb, wk2_sb, hts, db)
                vps = psmm.tile([128, TC], f32, tag="mm")
                mm8(vps, wv1_sb, wv2_sb, hts, db)
                rps = psmm.tile([128, TC], f32, tag="mm")
                mm8(rps, wr1_sb, wr2_sb, hts, db)
                if debug and s == 0:
                    for nme, ps in (('k', kps), ('v', vps), ('r', rps)):
                        dtmp = pd.tile([128, TC], f32, tag="d",
                                       name=f"dbg{nme}{db}")
                        nc.scalar.activation(dtmp[:], ps[:], AF.Copy,
                                             scale=1.0 / WS)
                        nc.sync.dma_start(dbg[nme][ch, db], dtmp[:])

                ek = pg.tile([128, TC], bf16, tag="gen")
                nc.scalar.activation(ek[:], kps[:], AF.Exp, scale=1.0 / WS)
                eku = pg.tile([128, TC], bf16, tag="gen")
                nc.scalar.activation(eku[:], kps[:], AF.Exp, scale=1.0 / WS,
                                     bias=eu_c[:, db:db + 1])
                vbf = pg.tile([128, TC], bf16, tag="gen")
                nc.scalar.activation(vbf[:], vps[:], AF.Copy, scale=1.0 / WS)
                enr = pg.tile([128, TC], bf16, tag="gen")
                nc.scalar.activation(enr[:], rps[:], AF.Exp, scale=-1.0 / WS)

                ekv = pg.tile([128, TC], bf16, tag="gen")
                nc.gpsimd.tensor_tensor(ekv[:], ek[:], vbf[:], AL.mult)
                ekuv = pg.tile([128, TC], bf16, tag="gen")
                nc.gpsimd.tensor_tensor(ekuv[:], eku[:], vbf[:], AL.mult)

                At = pga.tile([128, TC + 1], bf16, tag="genA")
                Bt = pga.tile([128, TC + 1], bf16, tag="genA")
                if ch == 0:
                    nc.vector.memset(At[:, 0:1], 0.0)
                    nc.vector.memset(Bt[:, 0:1], 0.0)
                else:
                    nc.vector.tensor_copy(At[:, 0:1], carryA[:, idx:idx + 1])
                    nc.vector.tensor_copy(Bt[:, 0:1], carryB[:, idx:idx + 1])
                lamb = lam_c[:, db:db + 1].broadcast_to((128, TC))
                nc.vector.tensor_tensor_scan(At[:, 1:TC + 1], lamb, ekv[:],
                                             At[:, 0:1], AL.mult, AL.add)
                nc.vector.tensor_tensor_scan(Bt[:, 1:TC + 1], lamb, ek[:],
                                             Bt[:, 0:1], AL.mult, AL.add)
                if ch < NCH - 1:
                    nc.vector.tensor_copy(carryA[:, idx:idx + 1],
                                          At[:, TC:TC + 1])
                    nc.vector.tensor_copy(carryB[:, idx:idx + 1],
                                          Bt[:, TC:TC + 1])

                num = pg.tile([128, TC], bf16, tag="gen")
                nc.vector.tensor_tensor(num[:], At[:, 0:TC], ekuv[:], AL.add)
                den = pg.tile([128, TC], bf16, tag="gen")
                nc.vector.tensor_tensor(den[:], Bt[:, 0:TC], eku[:], AL.add)
                enr1 = pg.tile([128, TC], bf16, tag="gen")
                nc.vector.tensor_scalar_add(enr1[:], enr[:], 1.0)
                den2 = pf32.tile([128, TC], f32, tag="f32s")
                nc.vector.tensor_tensor(den2[:], den[:], enr1[:], AL.mult)
                rec = pf32.tile([128, TC], f32, tag="f32s")
                nc.vector.reciprocal_approx_fast(rec[:], den2[:])
                nc.vector.tensor_tensor(sry_ts[db // 2][:, db % 2, :],
                                        num[:], rec[:], AL.mult)
                if debug and s == 0:
                    for nme, src in (('num', num), ('den', den)):
                        dtmp2 = pd.tile([128, TC], f32, tag="d",
                                        name=f"dbg{nme}{db}")
                        nc.vector.tensor_copy(dtmp2[:], src[:])
                        nc.sync.dma_start(dbg[nme][ch, db], dtmp2[:])

            x2_ts = []
            for j in range(PR):
                x2t = px2.tile([128, 2, TC], bf16, tag="x2")
                for i in range(2):
                    cb = 2 * j + i
                    xps = psmm.tile([128, TC], f32, tag="mm")
                    for jj in range(PR):
                        nc.tensor.matmul(
                            xps[:], wo_sb[jj][:, :, cb * 128:(cb + 1) * 128],
                            sry_ts[jj][:], start=(jj == 0), stop=(jj == PR - 1),
                            perf_mode=PM.DoubleRow)
                    xevac = pd.tile([128, TC], bf16, tag="d")
                    nc.scalar.activation(xevac[:], xps[:], AF.Copy,
                                         scale=1.0 / WS)
                    nc.vector.tensor_tensor(x2t[:, i, :], xevac[:],
                                            xts[j][:, i, :], AL.add)
                if debug and s == 0:
                    nc.sync.dma_start(dbg['sry'][ch, j], sry_ts[j][:])
                    nc.sync.dma_start(dbg['x2'][ch, j], x2t[:])
                x2_ts.append(x2t)
            return x2_ts

        def cm1(s, ch, x2_ts):
            mu_sb, rs_sb = ln_stats(x2_ts, g2_c, b2_c, use_gb2)
            h2ts = []
            for cb in range(CB):
                j, i = cb // 2, cb % 2
                h2t = ph2.tile([128, TC + 1], bf16, tag="h2")
                t = pd.tile([128, TC], bf16, tag="d")
                nc.vector.tensor_tensor(t[:], x2_ts[j][:, i, :], mu_sb[:],
                                        AL.subtract)
                nc.gpsimd.tensor_tensor(h2t[:, 1:TC + 1], t[:], rs_sb[:],
                                        AL.mult)
                if use_gb2:
                    nc.vector.tensor_scalar(h2t[:, 1:TC + 1], h2t[:, 1:TC + 1],
                                            g_col(g2_c, cb), g_col(b2_c, cb),
                                            AL.mult, AL.add)
                idx = s * CB + cb
                if ch == 0:
                    nc.vector.memset(h2t[:, 0:1], 0.0)
                else:
                    nc.vector.tensor_copy(h2t[:, 0:1], carryH2[:, idx:idx + 1])
                if ch < NCH - 1:
                    nc.vector.tensor_copy(carryH2[:, idx:idx + 1],
                                          h2t[:, TC:TC + 1])
                h2ts.append(h2t)
            mix_ts = [pmix.tile([128, 2, TC], f8, tag="mix", name=f"mix{_j}")
                      for _j in range(PR)]
            for cb in range(CB):
                idx = s * CB + cb
                dt = pd.tile([128, TC], bf16, tag="d")
                nc.vector.tensor_tensor(dt[:], h2ts[cb][:, 1:TC + 1],
                                        h2ts[cb][:, 0:TC], AL.subtract)
                nc.vector.scalar_tensor_tensor(
                    mix_ts[cb // 2][:, cb % 2, :], dt[:],
                    cmk_c[:, idx:idx + 1], h2ts[cb][:, 0:TC], AL.mult, AL.add)
            kk_ts = [pkk.tile([128, 2, TC], f8, tag="kk", name=f"kk{_j}")
                     for _j in range(HPR)]
            for hb in range(HB):
                wckt = pwck.tile([128, PR, 2, 128], f8, tag="wck")
                nc.sync.dma_start(wckt[:], wck8[hb])
                ckps = psmm.tile([128, TC], f32, tag="mm")
                for j in range(PR):
                    nc.tensor.matmul(ckps[:], wckt[:, j], mix_ts[j][:],
                                     start=(j == 0), stop=(j == PR - 1),
                                     perf_mode=PM.DoubleRow)
                relu = prelu.tile([128, TC], bf16, tag="relu")
                nc.scalar.activation(relu[:], ckps[:], AF.Relu, scale=1.0 / WS)
                if hb % 2 == 0:
                    nc.gpsimd.tensor_tensor(kk_ts[hb // 2][:, hb % 2, :],
                                            relu[:], relu[:], AL.mult)
                else:
                    nc.vector.tensor_tensor(kk_ts[hb // 2][:, hb % 2, :],
                                            relu[:], relu[:], AL.mult)
            if debug and s == 0:
                for j in range(PR):
                    nc.sync.dma_start(dbg['mix'][ch, j], mix_ts[j][:])
                for j in range(HPR):
                    nc.sync.dma_start(dbg['kk'][ch, j], kk_ts[j][:])
            return x2_ts, mix_ts, kk_ts

        def cm2(s, ch, st3):
            x2_ts, mix_ts, kk_ts = st3
            for cb in range(CB):
                crps = psmm.tile([128, TC], f32, tag="mm")
                for j in range(PR):
                    nc.tensor.matmul(crps[:], wcr_sb[cb][:, j], mix_ts[j][:],
                                     start=(j == 0), stop=(j == PR - 1),
                                     perf_mode=PM.DoubleRow)
                enz = pg.tile([128, TC], bf16, tag="gen")
                nc.scalar.activation(enz[:], crps[:], AF.Exp, scale=-1.0 / WS)
                enz1 = pf32.tile([128, TC], f32, tag="f32s")
                nc.vector.tensor_scalar(enz1[:], enz[:], WS, WS,
                                        AL.mult, AL.add)
                rec = pf32.tile([128, TC], f32, tag="f32s")
                nc.vector.reciprocal_approx_fast(rec[:], enz1[:])
                wcvt = pwcv.tile([128, HPR, 2, 128], f8, tag="wcv")
                nc.sync.dma_start(wcvt[:], wcv8[cb])
                kvps = psmm.tile([128, TC], f32, tag="mm")
                for j in range(HPR):
                    nc.tensor.matmul(kvps[:], wcvt[:, j], kk_ts[j][:],
                                     start=(j == 0), stop=(j == HPR - 1),
                                     perf_mode=PM.DoubleRow)
                t1 = pd.tile([128, TC], bf16, tag="d")
                nc.vector.tensor_tensor(t1[:], kvps[:], rec[:], AL.mult)
                outt = pout.tile([128, TC], f32, tag="out")
                nc.vector.tensor_tensor(outt[:], x2_ts[cb // 2][:, cb % 2, :],
                                        t1[:], AL.add)
                nc.sync.dma_start(
                    oct_[s, cb * 128:(cb + 1) * 128, ch * TC:(ch + 1) * TC],
                    outt[:])

        # software-pipelined emission as in the baseline:
        # tm1(u) -> cm2(prev) -> tm2(u) -> cm1(u)
        # ch-major: consecutive units belong to different sequences, so
        # their chains are independent and overlap on every engine.
        units = [(s, ch) for ch in range(NCH) for s in range(NSEQ)]
        cm1_st = {}
        prev = None
        for u in units:
            mark(f"tm1{u}")
            st_u = tm1_stats(*u)
            hts_u = tm1_apply(*u, st_u)
            if prev is not None:
                mark(f"cm2{prev}")
                cm2(*prev, cm1_st.pop(prev))
            mark(f"tm2{u}")
            x2_ts = tm2(*u, st_u[0], hts_u)
            mark(f"cm1{u}")
            cm1_st[u] = cm1(*u, x2_ts)
            prev = u
        mark(f"cm2{prev}")
        cm2(*prev, cm1_st.pop(prev))
        mark("end")

    nc.compile()
    return nc


def _prep_weights(inputs):
    """Host-side fp8 weight prep. Returns dict of arrays shared by all cores
    plus per-p folded TimeMix weights."""
    bf = ml_dtypes.bfloat16
    f8 = ml_dtypes.float8_e4m3

    def q8(a):
        return np.clip(a * WS, -240, 240).astype(f8)

    def fold_pair(W, m):
        # lhsT[c, d] = W[d, c] * m[c]; layout [j, p, i, d]
        WT = np.asarray(W, np.float32).T * m[:, None]
        return np.ascontiguousarray(
            WT.reshape(PR, 2, 128, C).transpose(0, 2, 1, 3))

    def plain_pair(W):
        WT = np.ascontiguousarray(np.asarray(W, np.float32).T)
        return np.ascontiguousarray(
            WT.reshape(PR, 2, 128, C).transpose(0, 2, 1, 3))

    out = {}
    mk = np.asarray(inputs['att_mix_k'], np.float32).reshape(P, C)
    mv = np.asarray(inputs['att_mix_v'], np.float32).reshape(P, C)
    mr = np.asarray(inputs['att_mix_r'], np.float32).reshape(P, C)
    for p in range(P):
        out[p] = {
            'wk1': q8(fold_pair(inputs['Wk'], mk[p])),
            'wk2': q8(fold_pair(inputs['Wk'], 1 - mk[p])),
            'wv1': q8(fold_pair(inputs['Wv'], mv[p])),
            'wv2': q8(fold_pair(inputs['Wv'], 1 - mv[p])),
            'wr1': q8(fold_pair(inputs['Wr'], mr[p])),
            'wr2': q8(fold_pair(inputs['Wr'], 1 - mr[p])),
        }
    shared = {'wo8': q8(plain_pair(inputs['Wo']))}
    # wck8[hb, p, j, i, dd] = Wck.T[(2j+i)*128+p, hb*128+dd] * WS
    WckT = np.asarray(inputs['Wck'], np.float32).T
    shared['wck8'] = q8(np.ascontiguousarray(
        WckT.reshape(PR, 2, 128, HB, 128).transpose(3, 2, 0, 1, 4)))
    WcvT = np.asarray(inputs['Wcv'], np.float32).T
    shared['wcv8'] = q8(np.ascontiguousarray(
        WcvT.reshape(HPR, 2, 128, CB, 128).transpose(3, 2, 0, 1, 4)))
    WcrT = np.asarray(inputs['Wcr'], np.float32).T
    shared['wcr8'] = q8(np.ascontiguousarray(
        WcrT.reshape(PR, 2, 128, CB, 128).transpose(3, 2, 0, 1, 4)))
    return out, shared


def kernel(**inputs):
    from concourse.bass_utils import run_bass_kernel_spmd

    x = np.asarray(inputs['x'], dtype=np.float32)
    g1 = np.asarray(inputs['ln1_g'], np.float32)
    b1 = np.asarray(inputs['ln1_b'], np.float32)
    g2 = np.asarray(inputs['ln2_g'], np.float32)
    b2 = np.asarray(inputs['ln2_b'], np.float32)
    use_gb1 = not (np.all(g1 == 1.0) and np.all(b1 == 0.0))
    use_gb2 = not (np.all(g2 == 1.0) and np.all(b2 == 0.0))
    cmk = np.asarray(inputs['cm_mix_k'], np.float32).reshape(P, C)
    cmr = np.asarray(inputs['cm_mix_r'], np.float32).reshape(P, C)
    assert np.array_equal(cmk, cmr), "kernel2 assumes cm_mix_k == cm_mix_r"

    debug = os.environ.get('RWKV_DEBUG', '0') == '1'
    key = (use_gb1, use_gb2, debug)
    if key not in _CACHE:
        _CACHE[key] = _build(use_gb1, use_gb2, debug)
    nc = _CACHE[key]

    bf = ml_dtypes.bfloat16
    lam = np.exp(-np.exp(np.asarray(inputs['time_decay'], np.float32)))
    # row 1 is raw u = time_first: it enters as the exp() bias on the device
    u = np.asarray(inputs['time_first'], np.float32)
    vec6 = np.stack([lam.astype(np.float32), u,
                     g1, b1, g2, b2]).astype(np.float32)

    perp, shared = _prep_weights(inputs)

    xf = x.reshape(P * B, T, C)
    in_maps = []
    for core in range(NCORES):
        seqs = [2 * core, 2 * core + 1]
        p = seqs[0] // B
        assert seqs[1] // B == p
        xcm = np.ascontiguousarray(xf[seqs].transpose(0, 2, 1)).astype(bf)
        in_maps.append({
            'xcm': xcm, 'vec6': vec6,
            'mixcm': np.stack([cmk[p], cmk[p]]).astype(np.float32),
            **perp[p], **shared,
        })

    trace = os.environ.get('RWKV_TRACE', '0') == '1'
    res = run_bass_kernel_spmd(nc, in_maps, list(range(NCORES)), trace=trace)
    global LAST_RUN_INFO
    LAST_RUN_INFO = res

    out = np.empty((P * B, T, C), np.float32)
    for core in range(NCORES):
        oc = res.results[core]['oct']
        out[2 * core] = oc[0].T
        out[2 * core + 1] = oc[1].T
    return out.reshape(P, B, T, C)


LAST_RUN_INFO = None


# revision 4
# speedup vs baseline: 1.0828x; 1.0428x over previous
# RWKV-v4 block (TimeMix WKV + ChannelMix) on 8 Trainium2 NeuronCores — v2.
#
# Sharding: data-parallel over the 16 (p, b) sequences -> 2 per core (both
# sequences of a core share the same p, so mix/fold constants are per-core).
#
# v2 strategy vs the bf16 baseline (924us cost-model):
# - All seven matmuls run as fp8e4m3 DoubleRow (2 k-tiles interleaved,
#   K=256/instr at 0.5 cyc/row): 4x fewer PE cycles than bf16.
# - TimeMix token-shift mixes are folded into the weights:
#   k = (Wk*diag(mk)) @ h + (Wk*diag(1-mk)) @ shift(h); h itself is written
#   directly in fp8 pair tiles [128, 2, TC+1] (col 0 = carry), so no mix
#   tiles and no shift ops exist on the vector engines for the k/v/r paths.
# - Weights are host-prescaled by WS=64 into the e4m3 normal range; every
#   inverse scale folds into an Activation scale= or the sigmoid chain.
# - WKV pipeline in bf16 (2x DVE mode on tensor_tensor), scans keep f32
#   lambda, reciprocals in f32 (hw requirement).
# - ChannelMix: cm_mix_k == cm_mix_r (runtime-checked) -> one shared mix
#   tile in fp8; Wck/Wcv are streamed per-unit in hb-/cb-blocked fp8 pair
#   layouts; relu on Act, squares split Pool/DVE -> fp8 (splitting the
#   32-op serial run across engines shortens the per-unit critical path).
import os
import numpy as np
import ml_dtypes

P, B, T, C = 2, 8, 1024, 1024
H = 4 * C
NCORES = 8
NSEQ = 2
TC = 512
NCH = T // TC      # 2
CB = C // 128      # 8
PR = CB // 2       # 4 channel-block pairs
HB = H // 128      # 32
HPR = HB // 2      # 16
EPS = 1e-5
WS = 64.0          # fp8 weight prescale

_CACHE = {}
PHASES = []


def _build(use_gb1, use_gb2, debug=False):
    import concourse.bass as bass
    import concourse.tile as tile
    from concourse import bacc, mybir

    f32 = mybir.dt.float32
    f32r = mybir.dt.float32r
    bf16 = mybir.dt.bfloat16
    f8 = mybir.dt.float8e4
    AL = mybir.AluOpType
    AF = mybir.ActivationFunctionType
    PM = mybir.MatmulPerfMode

    nc = bacc.Bacc()
    global PHASES
    PHASES = []

    def mark(label):
        # snapshot the next instruction index for phase attribution
        nm = nc.get_next_instruction_name()
        PHASES.append((int(nm.split('-')[1]), label))

    xcm = nc.dram_tensor("xcm", (NSEQ, C, T), bf16, kind="ExternalInput")
    # folded TimeMix weights: [PR, 128, 2, C] each
    wk1 = nc.dram_tensor("wk1", (PR, 128, 2, C), f8, kind="ExternalInput")
    wk2 = nc.dram_tensor("wk2", (PR, 128, 2, C), f8, kind="ExternalInput")
    wv1 = nc.dram_tensor("wv1", (PR, 128, 2, C), f8, kind="ExternalInput")
    wv2 = nc.dram_tensor("wv2", (PR, 128, 2, C), f8, kind="ExternalInput")
    wr1 = nc.dram_tensor("wr1", (PR, 128, 2, C), f8, kind="ExternalInput")
    wr2 = nc.dram_tensor("wr2", (PR, 128, 2, C), f8, kind="ExternalInput")
    wo8 = nc.dram_tensor("wo8", (PR, 128, 2, C), f8, kind="ExternalInput")
    wck8 = nc.dram_tensor("wck8", (HB, 128, PR, 2, 128), f8, kind="ExternalInput")
    wcv8 = nc.dram_tensor("wcv8", (CB, 128, HPR, 2, 128), f8, kind="ExternalInput")
    wcr8 = nc.dram_tensor("wcr8", (CB, 128, PR, 2, 128), f8, kind="ExternalInput")
    vec6 = nc.dram_tensor("vec6", (6, C), f32, kind="ExternalInput")
    mixcm = nc.dram_tensor("mixcm", (NSEQ, C), f32, kind="ExternalInput")
    oct_ = nc.dram_tensor("oct", (NSEQ, C, T), f32, kind="ExternalOutput")
    dbg = {}
    if debug:
        dbg['h'] = nc.dram_tensor("dbg_h", (NCH, PR, 128, 2, TC + 1), f8,
                                  kind="ExternalOutput")
        for n in ('k', 'v', 'r'):
            dbg[n] = nc.dram_tensor(f"dbg_{n}", (NCH, CB, 128, TC), f32,
                                    kind="ExternalOutput")
        dbg['sry'] = nc.dram_tensor("dbg_sry", (NCH, PR, 128, 2, TC), f8,
                                    kind="ExternalOutput")
        dbg['x2'] = nc.dram_tensor("dbg_x2", (NCH, PR, 128, 2, TC), bf16,
                                   kind="ExternalOutput")
        dbg['mix'] = nc.dram_tensor("dbg_mix", (NCH, PR, 128, 2, TC), f8,
                                    kind="ExternalOutput")
        dbg['kk'] = nc.dram_tensor("dbg_kk", (NCH, HPR, 128, 2, TC), f8,
                                   kind="ExternalOutput")
        dbg['num'] = nc.dram_tensor("dbg_num", (NCH, CB, 128, TC), f32,
                                    kind="ExternalOutput")
        dbg['den'] = nc.dram_tensor("dbg_den", (NCH, CB, 128, TC), f32,
                                    kind="ExternalOutput")

    from contextlib import ExitStack
    with ExitStack() as ctx:
        tc = ctx.enter_context(tile.TileContext(nc))
        pc = ctx.enter_context(tc.tile_pool(name="const", bufs=1))
        pw = ctx.enter_context(tc.tile_pool(name="wres", bufs=1))
        pwck = ctx.enter_context(tc.tile_pool(name="wckst", bufs=6))
        pwcv = ctx.enter_context(tc.tile_pool(name="wcvst", bufs=2))
        px = ctx.enter_context(tc.tile_pool(name="x", bufs=8))
        psq = ctx.enter_context(tc.tile_pool(name="sq", bufs=2))
        ph = ctx.enter_context(tc.tile_pool(name="h", bufs=8))
        pd = ctx.enter_context(tc.tile_pool(name="d", bufs=5))
        pstat = ctx.enter_context(tc.tile_pool(name="stat", bufs=2))
        pbc = ctx.enter_context(tc.tile_pool(name="bcc", bufs=4))
        pg = ctx.enter_context(tc.tile_pool(name="gen", bufs=12))
        pf32 = ctx.enter_context(tc.tile_pool(name="f32s", bufs=5))
        pga = ctx.enter_context(tc.tile_pool(name="genA", bufs=6))
        psry = ctx.enter_context(tc.tile_pool(name="sry", bufs=4))
        px2 = ctx.enter_context(tc.tile_pool(name="x2", bufs=6))
        ph2 = ctx.enter_context(tc.tile_pool(name="h2", bufs=8, ))
        pmix = ctx.enter_context(tc.tile_pool(name="mix", bufs=8))
        prelu = ctx.enter_context(tc.tile_pool(name="relu", bufs=3))
        pkk = ctx.enter_context(tc.tile_pool(name="kk", bufs=18))
        pout = ctx.enter_context(tc.tile_pool(name="out", bufs=2))
        psmm = ctx.enter_context(tc.tile_pool(name="psmm", bufs=6, space="PSUM"))
        psst = ctx.enter_context(tc.tile_pool(name="pss", bufs=2, space="PSUM"))

        # ---- constants ----
        invCb = pc.tile([128, 1], bf16, tag="invCb")
        nc.vector.memset(invCb[:], 1.0 / C)
        eps_t = pc.tile([128, 1], f32, tag="eps")
        nc.vector.memset(eps_t[:], EPS)
        negone_f = pc.tile([1, 1], f32, tag="negonef")
        nc.vector.memset(negone_f[:], -1.0)
        negone = pc.tile([1, 1], f32r, tag="negone")
        nc.scalar.copy(negone[:], negone_f[:])

        def colload(src_ap, ncol, dtype=f32, tag=None):
            t = pc.tile([128, ncol], dtype, tag=tag)
            nc.sync.dma_start(t[:], src_ap)
            return t

        lam_c = colload(vec6[0].rearrange("(j p) -> p j", p=128), CB, tag="lam")
        eu_c = colload(vec6[1].rearrange("(j p) -> p j", p=128), CB, tag="eu")
        g1_c = colload(vec6[2].rearrange("(j p) -> p j", p=128), CB, tag="g1")
        b1_c = colload(vec6[3].rearrange("(j p) -> p j", p=128), CB, tag="b1")
        g2_c = colload(vec6[4].rearrange("(j p) -> p j", p=128), CB, tag="g2")
        b2_c = colload(vec6[5].rearrange("(j p) -> p j", p=128), CB, tag="b2")
        cmk_c = colload(mixcm.rearrange("s (j p) -> p (s j)", p=128),
                        NSEQ * CB, tag="cmk")

        # carries (chunk -> chunk)
        carryH = pc.tile([128, NSEQ * CB], f8, tag="carryH")     # h pairs
        carryH2 = pc.tile([128, NSEQ * CB], bf16, tag="carryH2")  # h2
        carryA = pc.tile([128, NSEQ * CB], bf16, tag="carryA")
        carryB = pc.tile([128, NSEQ * CB], bf16, tag="carryB")

        # ---- resident weights ----
        def wload(src, tag):
            tiles = []
            for j in range(PR):
                t = pw.tile([128, 2, C], f8, tag=f"{tag}{j}")
                nc.sync.dma_start(t[:], src[j])
                tiles.append(t)
            return tiles

        wk1_sb = wload(wk1, "wk1")
        wk2_sb = wload(wk2, "wk2")
        wv1_sb = wload(wv1, "wv1")
        wv2_sb = wload(wv2, "wv2")
        wr1_sb = wload(wr1, "wr1")
        wr2_sb = wload(wr2, "wr2")
        wo_sb = wload(wo8, "wo")
        wcr_sb = []
        for cb in range(CB):
            t = pw.tile([128, PR, 2, 128], f8, tag=f"wcr{cb}")
            nc.sync.dma_start(t[:], wcr8[cb])
            wcr_sb.append(t)

        def ln_begin():
            s1 = psst.tile([1, TC], f32, tag="ss", name="s1")
            s2 = psst.tile([1, TC], f32, tag="ss", name="s2")
            return s1, s2

        def ln_feed(st, pair, j):
            """accumulate pair j's s1/s2 contributions (PE + one DVE sq)"""
            s1, s2 = st
            for i in range(2):
                nc.tensor.matmul(s1[:], invCb[:], pair[:, i, :],
                                 start=(j == 0 and i == 0),
                                 stop=(j == PR - 1 and i == 1))
                sq = psq.tile([128, TC], bf16, tag="sq")
                nc.vector.tensor_tensor(sq[:], pair[:, i, :],
                                        pair[:, i, :], AL.mult)
                nc.tensor.matmul(s2[:], invCb[:], sq[:],
                                 start=(j == 0 and i == 0), stop=False)

        def ln_finish(st):
            """var = s2 - mu^2 via an extra PSUM accumulation matmul so the
            stats chain stays PE<->Act only; returns broadcast (mu_sb, rs_sb)."""
            s1, s2 = st
            mu2 = pstat.tile([1, TC], f32r, tag="st")
            nc.scalar.activation(mu2[:], s1[:], AF.Square)
            mub = pstat.tile([1, TC], bf16, tag="st")
            nc.scalar.activation(mub[:], s1[:], AF.Copy)
            nc.tensor.matmul(s2[:], negone[:], mu2[:], start=False, stop=True)
            lnv = pstat.tile([1, TC], f32, tag="st")
            nc.scalar.activation(lnv[:], s2[:], AF.Ln, bias=eps_t[0:1, 0:1])
            rs = pstat.tile([1, TC], bf16, tag="st")
            nc.scalar.activation(rs[:], lnv[:], AF.Exp, scale=-0.5)
            mu_sb = pbc.tile([128, TC], bf16, tag="mu_sb")
            nc.gpsimd.partition_broadcast(mu_sb[:], mub[:], 128)
            rs_sb = pbc.tile([128, TC], bf16, tag="rs_sb")
            nc.gpsimd.partition_broadcast(rs_sb[:], rs[:], 128)
            return mu_sb, rs_sb

        def ln_stats(src_pairs, g_c, b_c, use_gb):
            st = ln_begin()
            for j in range(PR):
                ln_feed(st, src_pairs[j], j)
            return ln_finish(st)

        def tm1_stats(s, ch):
            xts = []
            xsrc = xcm[s].rearrange("(j i p) t -> j p i t", i=2, p=128)
            for j in range(PR):
                xt = px.tile([128, 2, TC], bf16, tag="x")
                nc.sync.dma_start(xt[:], xsrc[j, :, :, ch * TC:(ch + 1) * TC])
                xts.append(xt)
            mu_sb, rs_sb = ln_stats(xts, g1_c, b1_c, use_gb1)
            return xts, mu_sb, rs_sb

        def tm1_apply(s, ch, st):
            xts, mu_sb, rs_sb = st
            hts = []
            for j in range(PR):
                ht = ph.tile([128, 2, TC + 1], f8, tag="h")
                for i in range(2):
                    cb = 2 * j + i
                    t = pd.tile([128, TC], bf16, tag="d")
                    nc.vector.tensor_tensor(t[:], xts[j][:, i, :], mu_sb[:],
                                            AL.subtract)
                    if use_gb1:
                        tg = pd.tile([128, TC], bf16, tag="d")
                        nc.vector.tensor_tensor(tg[:], t[:], rs_sb[:], AL.mult)
                        nc.vector.tensor_scalar(
                            ht[:, i, 1:TC + 1], tg[:], g_col(g1_c, cb),
                            b_col(b1_c, cb), AL.mult, AL.add)
                    else:
                        nc.gpsimd.tensor_tensor(ht[:, i, 1:TC + 1], t[:],
                                                rs_sb[:], AL.mult)
                idx = s * CB + 2 * j
                if ch == 0:
                    nc.vector.memset(ht[:, :, 0:1], 0.0)
                else:
                    nc.vector.tensor_copy(ht[:, :, 0:1], carryH[:, idx:idx + 2])
                if ch < NCH - 1:
                    nc.vector.tensor_copy(carryH[:, idx:idx + 2],
                                          ht[:, :, TC:TC + 1])
                if debug and s == 0:
                    nc.sync.dma_start(dbg['h'][ch, j], ht[:])
                hts.append(ht)
            return hts

        def g_col(gc, cb):
            return gc[:, cb:cb + 1]

        def mm8(psum, w1_sb, w2_sb, hts, db):
            """accumulate W1 @ h + W2 @ shift(h) into psum (8 DR matmuls)"""
            lo, hi = db * 128, (db + 1) * 128
            for j in range(PR):
                nc.tensor.matmul(psum[:], w1_sb[j][:, :, lo:hi],
                                 hts[j][:, :, 1:TC + 1],
                                 start=(j == 0), stop=False,
                                 perf_mode=PM.DoubleRow)
            for j in range(PR):
                nc.tensor.matmul(psum[:], w2_sb[j][:, :, lo:hi],
                                 hts[j][:, :, 0:TC],
                                 start=False, stop=(j == PR - 1),
                                 perf_mode=PM.DoubleRow)

        def tm2(s, ch, xts, hts):
            sry_ts = [psry.tile([128, 2, TC], f8, tag="sry", name=f"sry{_j}")
                      for _j in range(PR)]
            for db in range(CB):
                idx = s * CB + db
                kps = psmm.tile([128, TC], f32, tag="mm")
                mm8(kps, wk1_sb, wk2_sb, hts, db)
                vps = psmm.tile([128, TC], f32, tag="mm")
                mm8(vps, wv1_sb, wv2_sb, hts, db)
                rps = psmm.tile([128, TC], f32, tag="mm")
                mm8(rps, wr1_sb, wr2_sb, hts, db)
                if debug and s == 0:
                    for nme, ps in (('k', kps), ('v', vps), ('r', rps)):
                        dtmp = pd.tile([128, TC], f32, tag="d",
                                       name=f"dbg{nme}{db}")
                        nc.scalar.activation(dtmp[:], ps[:], AF.Copy,
                                             scale=1.0 / WS)
                        nc.sync.dma_start(dbg[nme][ch, db], dtmp[:])

                ek = pg.tile([128, TC], bf16, tag="gen")
                nc.scalar.activation(ek[:], kps[:], AF.Exp, scale=1.0 / WS)
                eku = pg.tile([128, TC], bf16, tag="gen")
                nc.scalar.activation(eku[:], kps[:], AF.Exp, scale=1.0 / WS,
                                     bias=eu_c[:, db:db + 1])
                vbf = pg.tile([128, TC], bf16, tag="gen")
                nc.scalar.activation(vbf[:], vps[:], AF.Copy, scale=1.0 / WS)
                enr = pg.tile([128, TC], bf16, tag="gen")
                nc.scalar.activation(enr[:], rps[:], AF.Exp, scale=-1.0 / WS)

                ekv = pg.tile([128, TC], bf16, tag="gen")
                nc.gpsimd.tensor_tensor(ekv[:], ek[:], vbf[:], AL.mult)
                ekuv = pg.tile([128, TC], bf16, tag="gen")
                nc.gpsimd.tensor_tensor(ekuv[:], eku[:], vbf[:], AL.mult)

                At = pga.tile([128, TC + 1], bf16, tag="genA")
                Bt = pga.tile([128, TC + 1], bf16, tag="genA")
                if ch == 0:
                    nc.vector.memset(At[:, 0:1], 0.0)
                    nc.vector.memset(Bt[:, 0:1], 0.0)
                else:
                    nc.vector.tensor_copy(At[:, 0:1], carryA[:, idx:idx + 1])
                    nc.vector.tensor_copy(Bt[:, 0:1], carryB[:, idx:idx + 1])
                lamb = lam_c[:, db:db + 1].broadcast_to((128, TC))
                nc.vector.tensor_tensor_scan(At[:, 1:TC + 1], lamb, ekv[:],
                                             At[:, 0:1], AL.mult, AL.add)
                nc.vector.tensor_tensor_scan(Bt[:, 1:TC + 1], lamb, ek[:],
                                             Bt[:, 0:1], AL.mult, AL.add)
                if ch < NCH - 1:
                    nc.vector.tensor_copy(carryA[:, idx:idx + 1],
                                          At[:, TC:TC + 1])
                    nc.vector.tensor_copy(carryB[:, idx:idx + 1],
                                          Bt[:, TC:TC + 1])

                num = pg.tile([128, TC], bf16, tag="gen")
                nc.vector.tensor_tensor(num[:], At[:, 0:TC], ekuv[:], AL.add)
                den = pg.tile([128, TC], bf16, tag="gen")
                nc.vector.tensor_tensor(den[:], Bt[:, 0:TC], eku[:], AL.add)
                enr1 = pg.tile([128, TC], bf16, tag="gen")
                nc.vector.tensor_scalar_add(enr1[:], enr[:], 1.0)
                den2 = pf32.tile([128, TC], f32, tag="f32s")
                nc.vector.tensor_tensor(den2[:], den[:], enr1[:], AL.mult)
                rec = pf32.tile([128, TC], f32, tag="f32s")
                nc.vector.reciprocal_approx_fast(rec[:], den2[:])
                nc.vector.tensor_tensor(sry_ts[db // 2][:, db % 2, :],
                                        num[:], rec[:], AL.mult)
                if debug and s == 0:
                    for nme, src in (('num', num), ('den', den)):
                        dtmp2 = pd.tile([128, TC], f32, tag="d",
                                        name=f"dbg{nme}{db}")
                        nc.vector.tensor_copy(dtmp2[:], src[:])
                        nc.sync.dma_start(dbg[nme][ch, db], dtmp2[:])

            x2_ts = []
            for j in range(PR):
                x2t = px2.tile([128, 2, TC], bf16, tag="x2")
                for i in range(2):
                    cb = 2 * j + i
                    xps = psmm.tile([128, TC], f32, tag="mm")
                    for jj in range(PR):
                        nc.tensor.matmul(
                            xps[:], wo_sb[jj][:, :, cb * 128:(cb + 1) * 128],
                            sry_ts[jj][:], start=(jj == 0), stop=(jj == PR - 1),
                            perf_mode=PM.DoubleRow)
                    xevac = pd.tile([128, TC], bf16, tag="d")
                    nc.scalar.activation(xevac[:], xps[:], AF.Copy,
                                         scale=1.0 / WS)
                    nc.vector.tensor_tensor(x2t[:, i, :], xevac[:],
                                            xts[j][:, i, :], AL.add)
                if debug and s == 0:
                    nc.sync.dma_start(dbg['sry'][ch, j], sry_ts[j][:])
                    nc.sync.dma_start(dbg['x2'][ch, j], x2t[:])
                x2_ts.append(x2t)
            return x2_ts

        def cm1(s, ch, x2_ts):
            mu_sb, rs_sb = ln_stats(x2_ts, g2_c, b2_c, use_gb2)
            h2ts = []
            for cb in range(CB):
                j, i = cb // 2, cb % 2
                h2t = ph2.tile([128, TC + 1], bf16, tag="h2")
                t = pd.tile([128, TC], bf16, tag="d")
                nc.vector.tensor_tensor(t[:], x2_ts[j][:, i, :], mu_sb[:],
                                        AL.subtract)
                nc.gpsimd.tensor_tensor(h2t[:, 1:TC + 1], t[:], rs_sb[:],
                                        AL.mult)
                if use_gb2:
                    nc.vector.tensor_scalar(h2t[:, 1:TC + 1], h2t[:, 1:TC + 1],
                                            g_col(g2_c, cb), g_col(b2_c, cb),
                                            AL.mult, AL.add)
                idx = s * CB + cb
                if ch == 0:
                    nc.vector.memset(h2t[:, 0:1], 0.0)
                else:
                    nc.vector.tensor_copy(h2t[:, 0:1], carryH2[:, idx:idx + 1])
                if ch < NCH - 1:
                    nc.vector.tensor_copy(carryH2[:, idx:idx + 1],
                                          h2t[:, TC:TC + 1])
                h2ts.append(h2t)
            mix_ts = [pmix.tile([128, 2, TC], f8, tag="mix", name=f"mix{_j}")
                      for _j in range(PR)]
            for cb in range(CB):
                idx = s * CB + cb
                dt = pd.tile([128, TC], bf16, tag="d")
                nc.vector.tensor_tensor(dt[:], h2ts[cb][:, 1:TC + 1],
                                        h2ts[cb][:, 0:TC], AL.subtract)
                nc.vector.scalar_tensor_tensor(
                    mix_ts[cb // 2][:, cb % 2, :], dt[:],
                    cmk_c[:, idx:idx + 1], h2ts[cb][:, 0:TC], AL.mult, AL.add)
            kk_ts = [pkk.tile([128, 2, TC], f8, tag="kk", name=f"kk{_j}")
                     for _j in range(HPR)]
            for hb in range(HB):
                wckt = pwck.tile([128, PR, 2, 128], f8, tag="wck")
                nc.sync.dma_start(wckt[:], wck8[hb])
                ckps = psmm.tile([128, TC], f32, tag="mm")
                for j in range(PR):
                    nc.tensor.matmul(ckps[:], wckt[:, j], mix_ts[j][:],
                                     start=(j == 0), stop=(j == PR - 1),
                                     perf_mode=PM.DoubleRow)
                relu = prelu.tile([128, TC], bf16, tag="relu")
                nc.scalar.activation(relu[:], ckps[:], AF.Relu, scale=1.0 / WS)
                if hb % 2 == 0:
                    nc.gpsimd.tensor_tensor(kk_ts[hb // 2][:, hb % 2, :],
                                            relu[:], relu[:], AL.mult)
                else:
                    nc.vector.tensor_tensor(kk_ts[hb // 2][:, hb % 2, :],
                                            relu[:], relu[:], AL.mult)
            if debug and s == 0:
                for j in range(PR):
                    nc.sync.dma_start(dbg['mix'][ch, j], mix_ts[j][:])
                for j in range(HPR):
                    nc.sync.dma_start(dbg['kk'][ch, j], kk_ts[j][:])
            return x2_ts, mix_ts, kk_ts

        def cm2(s, ch, st3):
            x2_ts, mix_ts, kk_ts = st3
            for cb in range(CB):
                crps = psmm.tile([128, TC], f32, tag="mm")
                for j in range(PR):
                    nc.tensor.matmul(crps[:], wcr_sb[cb][:, j], mix_ts[j][:],
                                     start=(j == 0), stop=(j == PR - 1),
                                     perf_mode=PM.DoubleRow)
                enz = pg.tile([128, TC], bf16, tag="gen")
                nc.scalar.activation(enz[:], crps[:], AF.Exp, scale=-1.0 / WS)
                enz1 = pf32.tile([128, TC], f32, tag="f32s")
                nc.vector.tensor_scalar(enz1[:], enz[:], WS, WS,
                                        AL.mult, AL.add)
                rec = pf32.tile([128, TC], f32, tag="f32s")
                nc.vector.reciprocal_approx_fast(rec[:], enz1[:])
                wcvt = pwcv.tile([128, HPR, 2, 128], f8, tag="wcv")
                nc.sync.dma_start(wcvt[:], wcv8[cb])
                kvps = psmm.tile([128, TC], f32, tag="mm")
                for j in range(HPR):
                    nc.tensor.matmul(kvps[:], wcvt[:, j], kk_ts[j][:],
                                     start=(j == 0), stop=(j == HPR - 1),
                                     perf_mode=PM.DoubleRow)
                t1 = pd.tile([128, TC], bf16, tag="d")
                nc.vector.tensor_tensor(t1[:], kvps[:], rec[:], AL.mult)
                outt = pout.tile([128, TC], f32, tag="out")
                nc.vector.tensor_tensor(outt[:], x2_ts[cb // 2][:, cb % 2, :],
                                        t1[:], AL.add)
                nc.sync.dma_start(
                    oct_[s, cb * 128:(cb + 1) * 128, ch * TC:(ch + 1) * TC],
                    outt[:])

        # software-pipelined emission as in the baseline:
        # tm1(u) -> cm2(prev) -> tm2(u) -> cm1(u)
        # ch-major: consecutive units belong to different sequences, so
        # their chains are independent and overlap on every engine.
        units = [(s, ch) for ch in range(NCH) for s in range(NSEQ)]
        cm1_st = {}
        prev = None
        for u in units:
            mark(f"tm1{u}")
            st_u = tm1_stats(*u)
            hts_u = tm1_apply(*u, st_u)
            if prev is not None:
                mark(f"cm2{prev}")
                cm2(*prev, cm1_st.pop(prev))
            mark(f"tm2{u}")
            x2_ts = tm2(*u, st_u[0], hts_u)
            mark(f"cm1{u}")
            cm1_st[u] = cm1(*u, x2_ts)
            prev = u
        mark(f"cm2{prev}")
        cm2(*prev, cm1_st.pop(prev))
        mark("end")

    nc.compile()
    return nc


def _prep_weights(inputs):
    """Host-side fp8 weight prep. Returns dict of arrays shared by all cores
    plus per-p folded TimeMix weights."""
    bf = ml_dtypes.bfloat16
    f8 = ml_dtypes.float8_e4m3

    def q8(a):
        return np.clip(a * WS, -240, 240).astype(f8)

    def fold_pair(W, m):
        # lhsT[c, d] = W[d, c] * m[c]; layout [j, p, i, d]
        WT = np.asarray(W, np.float32).T * m[:, None]
        return np.ascontiguousarray(
            WT.reshape(PR, 2, 128, C).transpose(0, 2, 1, 3))

    def plain_pair(W):
        WT = np.ascontiguousarray(np.asarray(W, np.float32).T)
        return np.ascontiguousarray(
            WT.reshape(PR, 2, 128, C).transpose(0, 2, 1, 3))

    out = {}
    mk = np.asarray(inputs['att_mix_k'], np.float32).reshape(P, C)
    mv = np.asarray(inputs['att_mix_v'], np.float32).reshape(P, C)
    mr = np.asarray(inputs['att_mix_r'], np.float32).reshape(P, C)
    for p in range(P):
        out[p] = {
            'wk1': q8(fold_pair(inputs['Wk'], mk[p])),
            'wk2': q8(fold_pair(inputs['Wk'], 1 - mk[p])),
            'wv1': q8(fold_pair(inputs['Wv'], mv[p])),
            'wv2': q8(fold_pair(inputs['Wv'], 1 - mv[p])),
            'wr1': q8(fold_pair(inputs['Wr'], mr[p])),
            'wr2': q8(fold_pair(inputs['Wr'], 1 - mr[p])),
        }
    shared = {'wo8': q8(plain_pair(inputs['Wo']))}
    # wck8[hb, p, j, i, dd] = Wck.T[(2j+i)*128+p, hb*128+dd] * WS
    WckT = np.asarray(inputs['Wck'], np.float32).T
    shared['wck8'] = q8(np.ascontiguousarray(
        WckT.reshape(PR, 2, 128, HB, 128).transpose(3, 2, 0, 1, 4)))
    WcvT = np.asarray(inputs['Wcv'], np.float32).T
    shared['wcv8'] = q8(np.ascontiguousarray(
        WcvT.reshape(HPR, 2, 128, CB, 128).transpose(3, 2, 0, 1, 4)))
    WcrT = np.asarray(inputs['Wcr'], np.float32).T
    shared['wcr8'] = q8(np.ascontiguousarray(
        WcrT.reshape(PR, 2, 128, CB, 128).transpose(3, 2, 0, 1, 4)))
    return out, shared


def kernel(**inputs):
    from concourse.bass_utils import run_bass_kernel_spmd

    x = np.asarray(inputs['x'], dtype=np.float32)
    g1 = np.asarray(inputs['ln1_g'], np.float32)
    b1 = np.asarray(inputs['ln1_b'], np.float32)
    g2 = np.asarray(inputs['ln2_g'], np.float32)
    b2 = np.asarray(inputs['ln2_b'], np.float32)
    use_gb1 = not (np.all(g1 == 1.0) and np.all(b1 == 0.0))
    use_gb2 = not (np.all(g2 == 1.0) and np.all(b2 == 0.0))
    cmk = np.asarray(inputs['cm_mix_k'], np.float32).reshape(P, C)
    cmr = np.asarray(inputs['cm_mix_r'], np.float32).reshape(P, C)
    assert np.array_equal(cmk, cmr), "kernel2 assumes cm_mix_k == cm_mix_r"

    debug = os.environ.get('RWKV_DEBUG', '0') == '1'
    key = (use_gb1, use_gb2, debug)
    if key not in _CACHE:
        _CACHE[key] = _build(use_gb1, use_gb2, debug)
    nc = _CACHE[key]

    bf = ml_dtypes.bfloat16
    lam = np.exp(-np.exp(np.asarray(inputs['time_decay'], np.float32)))
    # row 1 is raw u = time_first: it enters as the exp() bias on the device
    u = np.asarray(inputs['time_first'], np.float32)
    vec6 = np.stack([lam.astype(np.float32), u,
                     g1, b1, g2, b2]).astype(np.float32)

    perp, shared = _prep_weights(inputs)

    xf = x.reshape(P * B, T, C)
    in_maps = []
    for core in range(NCORES):
        seqs = [2 * core, 2 * core + 1]
        p = seqs[0] // B
        assert seqs[1] // B == p
        xcm = np.ascontiguousarray(xf[seqs].transpose(0, 2, 1)).astype(bf)
        in_maps.append({
            'xcm': xcm, 'vec6': vec6,
            'mixcm': np.stack([cmk[p], cmk[p]]).astype(np.float32),
            **perp[p], **shared,
        })

    trace = os.environ.get('RWKV_TRACE', '0') == '1'
    res = run_bass_kernel_spmd(nc, in_maps, list(range(NCORES)), trace=trace)
    global LAST_RUN_INFO
    LAST_RUN_INFO = res

    out = np.empty((P * B, T, C), np.float32)
    for core in range(NCORES):
        oc = res.results[core]['oct']
        out[2 * core] = oc[0].T
        out[2 * core + 1] = oc[1].T
    return out.reshape(P, B, T, C)


LAST_RUN_INFO = None


# revision 5
# speedup vs baseline: 1.1049x; 1.0204x over previous
# RWKV-v4 block (TimeMix WKV + ChannelMix) on 8 Trainium2 NeuronCores — v2.
#
# Sharding: data-parallel over the 16 (p, b) sequences -> 2 per core (both
# sequences of a core share the same p, so mix/fold constants are per-core).
#
# v2 strategy vs the bf16 baseline (924us cost-model):
# - All seven matmuls run as fp8e4m3 DoubleRow (2 k-tiles interleaved,
#   K=256/instr at 0.5 cyc/row): 4x fewer PE cycles than bf16.
# - TimeMix token-shift mixes are folded into the weights:
#   k = (Wk*diag(mk)) @ h + (Wk*diag(1-mk)) @ shift(h); h itself is written
#   directly in fp8 pair tiles [128, 2, TC+1] (col 0 = carry), so no mix
#   tiles and no shift ops exist on the vector engines for the k/v/r paths.
# - Weights are host-prescaled by WS=64 into the e4m3 normal range; every
#   inverse scale folds into an Activation scale= or the sigmoid chain.
# - WKV pipeline in bf16 (2x DVE mode on tensor_tensor), scans keep f32
#   lambda, reciprocals in f32 (hw requirement).
# - ChannelMix: cm_mix_k == cm_mix_r (runtime-checked) -> one shared mix
#   tile in fp8; Wck/Wcv are streamed per-unit in hb-/cb-blocked fp8 pair
#   layouts; relu on Act, squares split Pool/DVE -> fp8 (splitting the
#   32-op serial run across engines shortens the per-unit critical path).
import os
import numpy as np
import ml_dtypes

P, B, T, C = 2, 8, 1024, 1024
H = 4 * C
NCORES = 8
NSEQ = 2
TC = 512
NCH = T // TC      # 2
CB = C // 128      # 8
PR = CB // 2       # 4 channel-block pairs
HB = H // 128      # 32
HPR = HB // 2      # 16
EPS = 1e-5
WS = 64.0          # fp8 weight prescale

_CACHE = {}
PHASES = []


def _build(use_gb1, use_gb2, debug=False):
    import concourse.bass as bass
    import concourse.tile as tile
    from concourse import bacc, mybir

    f32 = mybir.dt.float32
    f32r = mybir.dt.float32r
    bf16 = mybir.dt.bfloat16
    f8 = mybir.dt.float8e4
    AL = mybir.AluOpType
    AF = mybir.ActivationFunctionType
    PM = mybir.MatmulPerfMode

    nc = bacc.Bacc()
    global PHASES
    PHASES = []

    def mark(label):
        # snapshot the next instruction index for phase attribution
        nm = nc.get_next_instruction_name()
        PHASES.append((int(nm.split('-')[1]), label))

    xcm = nc.dram_tensor("xcm", (NSEQ, C, T), bf16, kind="ExternalInput")
    # folded TimeMix weights: [PR, 128, 2, C] each
    wk1 = nc.dram_tensor("wk1", (PR, 128, 2, C), f8, kind="ExternalInput")
    wk2 = nc.dram_tensor("wk2", (PR, 128, 2, C), f8, kind="ExternalInput")
    wv1 = nc.dram_tensor("wv1", (PR, 128, 2, C), f8, kind="ExternalInput")
    wv2 = nc.dram_tensor("wv2", (PR, 128, 2, C), f8, kind="ExternalInput")
    wr1 = nc.dram_tensor("wr1", (PR, 128, 2, C), f8, kind="ExternalInput")
    wr2 = nc.dram_tensor("wr2", (PR, 128, 2, C), f8, kind="ExternalInput")
    wo8 = nc.dram_tensor("wo8", (PR, 128, 2, C), f8, kind="ExternalInput")
    wck8 = nc.dram_tensor("wck8", (HB, 128, PR, 2, 128), f8, kind="ExternalInput")
    wcv8 = nc.dram_tensor("wcv8", (CB, 128, HPR, 2, 128), f8, kind="ExternalInput")
    wcr8 = nc.dram_tensor("wcr8", (CB, 128, PR, 2, 128), f8, kind="ExternalInput")
    vec6 = nc.dram_tensor("vec6", (6, C), f32, kind="ExternalInput")
    mixcm = nc.dram_tensor("mixcm", (NSEQ, C), f32, kind="ExternalInput")
    oct_ = nc.dram_tensor("oct", (NSEQ, C, T), f32, kind="ExternalOutput")
    dbg = {}
    if debug:
        dbg['h'] = nc.dram_tensor("dbg_h", (NCH, PR, 128, 2, TC + 1), f8,
                                  kind="ExternalOutput")
        for n in ('k', 'v', 'r'):
            dbg[n] = nc.dram_tensor(f"dbg_{n}", (NCH, CB, 128, TC), f32,
                                    kind="ExternalOutput")
        dbg['sry'] = nc.dram_tensor("dbg_sry", (NCH, PR, 128, 2, TC), f8,
                                    kind="ExternalOutput")
        dbg['x2'] = nc.dram_tensor("dbg_x2", (NCH, PR, 128, 2, TC), bf16,
                                   kind="ExternalOutput")
        dbg['mix'] = nc.dram_tensor("dbg_mix", (NCH, PR, 128, 2, TC), f8,
                                    kind="ExternalOutput")
        dbg['kk'] = nc.dram_tensor("dbg_kk", (NCH, HPR, 128, 2, TC), f8,
                                   kind="ExternalOutput")
        dbg['num'] = nc.dram_tensor("dbg_num", (NCH, CB, 128, TC), f32,
                                    kind="ExternalOutput")
        dbg['den'] = nc.dram_tensor("dbg_den", (NCH, CB, 128, TC), f32,
                                    kind="ExternalOutput")

    from contextlib import ExitStack
    with ExitStack() as ctx:
        tc = ctx.enter_context(tile.TileContext(nc))
        pc = ctx.enter_context(tc.tile_pool(name="const", bufs=1))
        pw = ctx.enter_context(tc.tile_pool(name="wres", bufs=1))
        pwck = ctx.enter_context(tc.tile_pool(name="wckst", bufs=6))
        pwcv = ctx.enter_context(tc.tile_pool(name="wcvst", bufs=2))
        px = ctx.enter_context(tc.tile_pool(name="x", bufs=8))
        psq = ctx.enter_context(tc.tile_pool(name="sq", bufs=2))
        ph = ctx.enter_context(tc.tile_pool(name="h", bufs=8))
        pd = ctx.enter_context(tc.tile_pool(name="d", bufs=5))
        pstat = ctx.enter_context(tc.tile_pool(name="stat", bufs=2))
        pbc = ctx.enter_context(tc.tile_pool(name="bcc", bufs=4))
        pg = ctx.enter_context(tc.tile_pool(name="gen", bufs=12))
        pf32 = ctx.enter_context(tc.tile_pool(name="f32s", bufs=5))
        pga = ctx.enter_context(tc.tile_pool(name="genA", bufs=6))
        psry = ctx.enter_context(tc.tile_pool(name="sry", bufs=4))
        px2 = ctx.enter_context(tc.tile_pool(name="x2", bufs=6))
        ph2 = ctx.enter_context(tc.tile_pool(name="h2", bufs=8, ))
        pmix = ctx.enter_context(tc.tile_pool(name="mix", bufs=8))
        prelu = ctx.enter_context(tc.tile_pool(name="relu", bufs=3))
        pkk = ctx.enter_context(tc.tile_pool(name="kk", bufs=18))
        pout = ctx.enter_context(tc.tile_pool(name="out", bufs=2))
        psmm = ctx.enter_context(tc.tile_pool(name="psmm", bufs=6, space="PSUM"))
        psst = ctx.enter_context(tc.tile_pool(name="pss", bufs=2, space="PSUM"))

        # ---- constants ----
        invCb = pc.tile([128, 1], bf16, tag="invCb")
        nc.vector.memset(invCb[:], 1.0 / C)
        eps_t = pc.tile([128, 1], f32, tag="eps")
        nc.vector.memset(eps_t[:], EPS)
        negone_f = pc.tile([1, 1], f32, tag="negonef")
        nc.vector.memset(negone_f[:], -1.0)
        negone = pc.tile([1, 1], f32r, tag="negone")
        nc.scalar.copy(negone[:], negone_f[:])

        def colload(src_ap, ncol, dtype=f32, tag=None):
            t = pc.tile([128, ncol], dtype, tag=tag)
            nc.sync.dma_start(t[:], src_ap)
            return t

        lam_c = colload(vec6[0].rearrange("(j p) -> p j", p=128), CB, tag="lam")
        eu_c = colload(vec6[1].rearrange("(j p) -> p j", p=128), CB, tag="eu")
        g1_c = colload(vec6[2].rearrange("(j p) -> p j", p=128), CB, tag="g1")
        b1_c = colload(vec6[3].rearrange("(j p) -> p j", p=128), CB, tag="b1")
        g2_c = colload(vec6[4].rearrange("(j p) -> p j", p=128), CB, tag="g2")
        b2_c = colload(vec6[5].rearrange("(j p) -> p j", p=128), CB, tag="b2")
        cmk_c = colload(mixcm.rearrange("s (j p) -> p (s j)", p=128),
                        NSEQ * CB, tag="cmk")

        # carries (chunk -> chunk)
        carryH = pc.tile([128, NSEQ * CB], f8, tag="carryH")     # h pairs
        carryH2 = pc.tile([128, NSEQ * CB], bf16, tag="carryH2")  # h2
        carryA = pc.tile([128, NSEQ * CB], bf16, tag="carryA")
        carryB = pc.tile([128, NSEQ * CB], bf16, tag="carryB")

        # ---- resident weights ----
        def wload(src, tag):
            tiles = []
            for j in range(PR):
                t = pw.tile([128, 2, C], f8, tag=f"{tag}{j}")
                nc.sync.dma_start(t[:], src[j])
                tiles.append(t)
            return tiles

        wk1_sb, wk2_sb, wv1_sb, wv2_sb = [], [], [], []
        wr1_sb, wr2_sb, wo_sb, wcr_sb = [], [], [], []

        def load_weights():
            # emitted AFTER unit 0's tm1 so the first x DMAs are not queued
            # behind 7MB of weight traffic on the SP/HWDGE path
            for lst, src, tag in ((wk1_sb, wk1, "wk1"), (wk2_sb, wk2, "wk2"),
                                  (wv1_sb, wv1, "wv1"), (wv2_sb, wv2, "wv2"),
                                  (wr1_sb, wr1, "wr1"), (wr2_sb, wr2, "wr2"),
                                  (wo_sb, wo8, "wo")):
                lst.extend(wload(src, tag))
            for cb in range(CB):
                t = pw.tile([128, PR, 2, 128], f8, tag=f"wcr{cb}",
                            name=f"wcr{cb}")
                nc.sync.dma_start(t[:], wcr8[cb])
                wcr_sb.append(t)

        def ln_begin():
            s1 = psst.tile([1, TC], f32, tag="ss", name="s1")
            s2 = psst.tile([1, TC], f32, tag="ss", name="s2")
            return s1, s2

        def ln_feed(st, pair, j):
            """accumulate pair j's s1/s2 contributions (PE + one DVE sq)"""
            s1, s2 = st
            for i in range(2):
                nc.tensor.matmul(s1[:], invCb[:], pair[:, i, :],
                                 start=(j == 0 and i == 0),
                                 stop=(j == PR - 1 and i == 1))
                sq = psq.tile([128, TC], bf16, tag="sq")
                nc.vector.tensor_tensor(sq[:], pair[:, i, :],
                                        pair[:, i, :], AL.mult)
                nc.tensor.matmul(s2[:], invCb[:], sq[:],
                                 start=(j == 0 and i == 0), stop=False)

        def ln_finish(st):
            """var = s2 - mu^2 via an extra PSUM accumulation matmul so the
            stats chain stays PE<->Act only; returns broadcast (mu_sb, rs_sb)."""
            s1, s2 = st
            mu2 = pstat.tile([1, TC], f32r, tag="st")
            nc.scalar.activation(mu2[:], s1[:], AF.Square)
            mub = pstat.tile([1, TC], bf16, tag="st")
            nc.scalar.activation(mub[:], s1[:], AF.Copy)
            nc.tensor.matmul(s2[:], negone[:], mu2[:], start=False, stop=True)
            lnv = pstat.tile([1, TC], f32, tag="st")
            nc.scalar.activation(lnv[:], s2[:], AF.Ln, bias=eps_t[0:1, 0:1])
            rs = pstat.tile([1, TC], bf16, tag="st")
            nc.scalar.activation(rs[:], lnv[:], AF.Exp, scale=-0.5)
            mu_sb = pbc.tile([128, TC], bf16, tag="mu_sb")
            nc.gpsimd.partition_broadcast(mu_sb[:], mub[:], 128)
            rs_sb = pbc.tile([128, TC], bf16, tag="rs_sb")
            nc.gpsimd.partition_broadcast(rs_sb[:], rs[:], 128)
            return mu_sb, rs_sb

        def ln_stats(src_pairs, g_c, b_c, use_gb):
            st = ln_begin()
            for j in range(PR):
                ln_feed(st, src_pairs[j], j)
            return ln_finish(st)

        def tm1_stats(s, ch):
            xts = []
            xsrc = xcm[s].rearrange("(j i p) t -> j p i t", i=2, p=128)
            for j in range(PR):
                xt = px.tile([128, 2, TC], bf16, tag="x")
                nc.sync.dma_start(xt[:], xsrc[j, :, :, ch * TC:(ch + 1) * TC])
                xts.append(xt)
            mu_sb, rs_sb = ln_stats(xts, g1_c, b1_c, use_gb1)
            return xts, mu_sb, rs_sb

        def tm1_apply(s, ch, st):
            xts, mu_sb, rs_sb = st
            hts = []
            for j in range(PR):
                ht = ph.tile([128, 2, TC + 1], f8, tag="h")
                for i in range(2):
                    cb = 2 * j + i
                    t = pd.tile([128, TC], bf16, tag="d")
                    nc.vector.tensor_tensor(t[:], xts[j][:, i, :], mu_sb[:],
                                            AL.subtract)
                    if use_gb1:
                        tg = pd.tile([128, TC], bf16, tag="d")
                        nc.vector.tensor_tensor(tg[:], t[:], rs_sb[:], AL.mult)
                        nc.vector.tensor_scalar(
                            ht[:, i, 1:TC + 1], tg[:], g_col(g1_c, cb),
                            b_col(b1_c, cb), AL.mult, AL.add)
                    else:
                        nc.gpsimd.tensor_tensor(ht[:, i, 1:TC + 1], t[:],
                                                rs_sb[:], AL.mult)
                idx = s * CB + 2 * j
                if ch == 0:
                    nc.vector.memset(ht[:, :, 0:1], 0.0)
                else:
                    nc.vector.tensor_copy(ht[:, :, 0:1], carryH[:, idx:idx + 2])
                if ch < NCH - 1:
                    nc.vector.tensor_copy(carryH[:, idx:idx + 2],
                                          ht[:, :, TC:TC + 1])
                if debug and s == 0:
                    nc.sync.dma_start(dbg['h'][ch, j], ht[:])
                hts.append(ht)
            return hts

        def g_col(gc, cb):
            return gc[:, cb:cb + 1]

        def mm8(psum, w1_sb, w2_sb, hts, db):
            """accumulate W1 @ h + W2 @ shift(h) into psum (8 DR matmuls)"""
            lo, hi = db * 128, (db + 1) * 128
            for j in range(PR):
                nc.tensor.matmul(psum[:], w1_sb[j][:, :, lo:hi],
                                 hts[j][:, :, 1:TC + 1],
                                 start=(j == 0), stop=False,
                                 perf_mode=PM.DoubleRow)
            for j in range(PR):
                nc.tensor.matmul(psum[:], w2_sb[j][:, :, lo:hi],
                                 hts[j][:, :, 0:TC],
                                 start=False, stop=(j == PR - 1),
                                 perf_mode=PM.DoubleRow)

        def tm2(s, ch, xts, hts):
            sry_ts = [psry.tile([128, 2, TC], f8, tag="sry", name=f"sry{_j}")
                      for _j in range(PR)]
            for db in range(CB):
                idx = s * CB + db
                kps = psmm.tile([128, TC], f32, tag="mm")
                mm8(kps, wk1_sb, wk2_sb, hts, db)
                vps = psmm.tile([128, TC], f32, tag="mm")
                mm8(vps, wv1_sb, wv2_sb, hts, db)
                rps = psmm.tile([128, TC], f32, tag="mm")
                mm8(rps, wr1_sb, wr2_sb, hts, db)
                if debug and s == 0:
                    for nme, ps in (('k', kps), ('v', vps), ('r', rps)):
                        dtmp = pd.tile([128, TC], f32, tag="d",
                                       name=f"dbg{nme}{db}")
                        nc.scalar.activation(dtmp[:], ps[:], AF.Copy,
                                             scale=1.0 / WS)
                        nc.sync.dma_start(dbg[nme][ch, db], dtmp[:])

                ek = pg.tile([128, TC], bf16, tag="gen")
                nc.scalar.activation(ek[:], kps[:], AF.Exp, scale=1.0 / WS)
                eku = pg.tile([128, TC], bf16, tag="gen")
                nc.scalar.activation(eku[:], kps[:], AF.Exp, scale=1.0 / WS,
                                     bias=eu_c[:, db:db + 1])
                vbf = pg.tile([128, TC], bf16, tag="gen")
                nc.scalar.activation(vbf[:], vps[:], AF.Copy, scale=1.0 / WS)
                enr = pg.tile([128, TC], bf16, tag="gen")
                nc.scalar.activation(enr[:], rps[:], AF.Exp, scale=-1.0 / WS)

                ekv = pg.tile([128, TC], bf16, tag="gen")
                nc.gpsimd.tensor_tensor(ekv[:], ek[:], vbf[:], AL.mult)
                ekuv = pg.tile([128, TC], bf16, tag="gen")
                nc.gpsimd.tensor_tensor(ekuv[:], eku[:], vbf[:], AL.mult)

                At = pga.tile([128, TC + 1], bf16, tag="genA")
                Bt = pga.tile([128, TC + 1], bf16, tag="genA")
                if ch == 0:
                    nc.vector.memset(At[:, 0:1], 0.0)
                    nc.vector.memset(Bt[:, 0:1], 0.0)
                else:
                    nc.vector.tensor_copy(At[:, 0:1], carryA[:, idx:idx + 1])
                    nc.vector.tensor_copy(Bt[:, 0:1], carryB[:, idx:idx + 1])
                lamb = lam_c[:, db:db + 1].broadcast_to((128, TC))
                nc.vector.tensor_tensor_scan(At[:, 1:TC + 1], lamb, ekv[:],
                                             At[:, 0:1], AL.mult, AL.add)
                nc.vector.tensor_tensor_scan(Bt[:, 1:TC + 1], lamb, ek[:],
                                             Bt[:, 0:1], AL.mult, AL.add)
                if ch < NCH - 1:
                    nc.vector.tensor_copy(carryA[:, idx:idx + 1],
                                          At[:, TC:TC + 1])
                    nc.vector.tensor_copy(carryB[:, idx:idx + 1],
                                          Bt[:, TC:TC + 1])

                num = pg.tile([128, TC], bf16, tag="gen")
                nc.vector.tensor_tensor(num[:], At[:, 0:TC], ekuv[:], AL.add)
                den = pg.tile([128, TC], bf16, tag="gen")
                nc.vector.tensor_tensor(den[:], Bt[:, 0:TC], eku[:], AL.add)
                enr1 = pg.tile([128, TC], bf16, tag="gen")
                nc.vector.tensor_scalar_add(enr1[:], enr[:], 1.0)
                den2 = pf32.tile([128, TC], f32, tag="f32s")
                nc.vector.tensor_tensor(den2[:], den[:], enr1[:], AL.mult)
                rec = pf32.tile([128, TC], f32, tag="f32s")
                nc.vector.reciprocal_approx_fast(rec[:], den2[:])
                nc.vector.tensor_tensor(sry_ts[db // 2][:, db % 2, :],
                                        num[:], rec[:], AL.mult)
                if debug and s == 0:
                    for nme, src in (('num', num), ('den', den)):
                        dtmp2 = pd.tile([128, TC], f32, tag="d",
                                        name=f"dbg{nme}{db}")
                        nc.vector.tensor_copy(dtmp2[:], src[:])
                        nc.sync.dma_start(dbg[nme][ch, db], dtmp2[:])

            x2_ts = []
            for j in range(PR):
                x2t = px2.tile([128, 2, TC], bf16, tag="x2")
                for i in range(2):
                    cb = 2 * j + i
                    xps = psmm.tile([128, TC], f32, tag="mm")
                    for jj in range(PR):
                        nc.tensor.matmul(
                            xps[:], wo_sb[jj][:, :, cb * 128:(cb + 1) * 128],
                            sry_ts[jj][:], start=(jj == 0), stop=(jj == PR - 1),
                            perf_mode=PM.DoubleRow)
                    xevac = pd.tile([128, TC], bf16, tag="d")
                    nc.scalar.activation(xevac[:], xps[:], AF.Copy,
                                         scale=1.0 / WS)
                    nc.vector.tensor_tensor(x2t[:, i, :], xevac[:],
                                            xts[j][:, i, :], AL.add)
                if debug and s == 0:
                    nc.sync.dma_start(dbg['sry'][ch, j], sry_ts[j][:])
                    nc.sync.dma_start(dbg['x2'][ch, j], x2t[:])
                x2_ts.append(x2t)
            return x2_ts

        def cm1(s, ch, x2_ts):
            mu_sb, rs_sb = ln_stats(x2_ts, g2_c, b2_c, use_gb2)
            h2ts = []
            for cb in range(CB):
                j, i = cb // 2, cb % 2
                h2t = ph2.tile([128, TC + 1], bf16, tag="h2")
                t = pd.tile([128, TC], bf16, tag="d")
                nc.vector.tensor_tensor(t[:], x2_ts[j][:, i, :], mu_sb[:],
                                        AL.subtract)
                nc.gpsimd.tensor_tensor(h2t[:, 1:TC + 1], t[:], rs_sb[:],
                                        AL.mult)
                if use_gb2:
                    nc.vector.tensor_scalar(h2t[:, 1:TC + 1], h2t[:, 1:TC + 1],
                                            g_col(g2_c, cb), g_col(b2_c, cb),
                                            AL.mult, AL.add)
                idx = s * CB + cb
                if ch == 0:
                    nc.vector.memset(h2t[:, 0:1], 0.0)
                else:
                    nc.vector.tensor_copy(h2t[:, 0:1], carryH2[:, idx:idx + 1])
                if ch < NCH - 1:
                    nc.vector.tensor_copy(carryH2[:, idx:idx + 1],
                                          h2t[:, TC:TC + 1])
                h2ts.append(h2t)
            mix_ts = [pmix.tile([128, 2, TC], f8, tag="mix", name=f"mix{_j}")
                      for _j in range(PR)]
            for cb in range(CB):
                idx = s * CB + cb
                dt = pd.tile([128, TC], bf16, tag="d")
                nc.vector.tensor_tensor(dt[:], h2ts[cb][:, 1:TC + 1],
                                        h2ts[cb][:, 0:TC], AL.subtract)
                nc.vector.scalar_tensor_tensor(
                    mix_ts[cb // 2][:, cb % 2, :], dt[:],
                    cmk_c[:, idx:idx + 1], h2ts[cb][:, 0:TC], AL.mult, AL.add)
            kk_ts = [pkk.tile([128, 2, TC], f8, tag="kk", name=f"kk{_j}")
                     for _j in range(HPR)]
            for hb in range(HB):
                wckt = pwck.tile([128, PR, 2, 128], f8, tag="wck")
                nc.sync.dma_start(wckt[:], wck8[hb])
                ckps = psmm.tile([128, TC], f32, tag="mm")
                for j in range(PR):
                    nc.tensor.matmul(ckps[:], wckt[:, j], mix_ts[j][:],
                                     start=(j == 0), stop=(j == PR - 1),
                                     perf_mode=PM.DoubleRow)
                relu = prelu.tile([128, TC], bf16, tag="relu")
                nc.scalar.activation(relu[:], ckps[:], AF.Relu, scale=1.0 / WS)
                if hb % 2 == 0:
                    nc.gpsimd.tensor_tensor(kk_ts[hb // 2][:, hb % 2, :],
                                            relu[:], relu[:], AL.mult)
                else:
                    nc.vector.tensor_tensor(kk_ts[hb // 2][:, hb % 2, :],
                                            relu[:], relu[:], AL.mult)
            if debug and s == 0:
                for j in range(PR):
                    nc.sync.dma_start(dbg['mix'][ch, j], mix_ts[j][:])
                for j in range(HPR):
                    nc.sync.dma_start(dbg['kk'][ch, j], kk_ts[j][:])
            return x2_ts, mix_ts, kk_ts

        def cm2(s, ch, st3):
            x2_ts, mix_ts, kk_ts = st3
            for cb in range(CB):
                crps = psmm.tile([128, TC], f32, tag="mm")
                for j in range(PR):
                    nc.tensor.matmul(crps[:], wcr_sb[cb][:, j], mix_ts[j][:],
                                     start=(j == 0), stop=(j == PR - 1),
                                     perf_mode=PM.DoubleRow)
                enz = pg.tile([128, TC], bf16, tag="gen")
                nc.scalar.activation(enz[:], crps[:], AF.Exp, scale=-1.0 / WS)
                enz1 = pf32.tile([128, TC], f32, tag="f32s")
                nc.vector.tensor_scalar(enz1[:], enz[:], WS, WS,
                                        AL.mult, AL.add)
                rec = pf32.tile([128, TC], f32, tag="f32s")
                nc.vector.reciprocal_approx_fast(rec[:], enz1[:])
                wcvt = pwcv.tile([128, HPR, 2, 128], f8, tag="wcv")
                nc.sync.dma_start(wcvt[:], wcv8[cb])
                kvps = psmm.tile([128, TC], f32, tag="mm")
                for j in range(HPR):
                    nc.tensor.matmul(kvps[:], wcvt[:, j], kk_ts[j][:],
                                     start=(j == 0), stop=(j == HPR - 1),
                                     perf_mode=PM.DoubleRow)
                t1 = pd.tile([128, TC], bf16, tag="d")
                nc.vector.tensor_tensor(t1[:], kvps[:], rec[:], AL.mult)
                outt = pout.tile([128, TC], f32, tag="out")
                nc.vector.tensor_tensor(outt[:], x2_ts[cb // 2][:, cb % 2, :],
                                        t1[:], AL.add)
                nc.sync.dma_start(
                    oct_[s, cb * 128:(cb + 1) * 128, ch * TC:(ch + 1) * TC],
                    outt[:])

        # software-pipelined emission as in the baseline:
        # tm1(u) -> cm2(prev) -> tm2(u) -> cm1(u)
        # ch-major: consecutive units belong to different sequences, so
        # their chains are independent and overlap on every engine.
        units = [(s, ch) for ch in range(NCH) for s in range(NSEQ)]
        cm1_st = {}
        prev = None
        u0 = units[0]
        mark(f"tm1{u0}")
        st0 = tm1_stats(*u0)
        hts0 = tm1_apply(*u0, st0)
        load_weights()
        for idx, u in enumerate(units):
            if idx == 0:
                st_u, hts_u = st0, hts0
            else:
                mark(f"tm1{u}")
                st_u = tm1_stats(*u)
                hts_u = tm1_apply(*u, st_u)
            if prev is not None:
                mark(f"cm2{prev}")
                cm2(*prev, cm1_st.pop(prev))
            mark(f"tm2{u}")
            x2_ts = tm2(*u, st_u[0], hts_u)
            mark(f"cm1{u}")
            cm1_st[u] = cm1(*u, x2_ts)
            prev = u
        mark(f"cm2{prev}")
        cm2(*prev, cm1_st.pop(prev))
        mark("end")

    nc.compile()
    return nc


def _prep_weights(inputs):
    """Host-side fp8 weight prep. Returns dict of arrays shared by all cores
    plus per-p folded TimeMix weights."""
    bf = ml_dtypes.bfloat16
    f8 = ml_dtypes.float8_e4m3

    def q8(a):
        return np.clip(a * WS, -240, 240).astype(f8)

    def fold_pair(W, m):
        # lhsT[c, d] = W[d, c] * m[c]; layout [j, p, i, d]
        WT = np.asarray(W, np.float32).T * m[:, None]
        return np.ascontiguousarray(
            WT.reshape(PR, 2, 128, C).transpose(0, 2, 1, 3))

    def plain_pair(W):
        WT = np.ascontiguousarray(np.asarray(W, np.float32).T)
        return np.ascontiguousarray(
            WT.reshape(PR, 2, 128, C).transpose(0, 2, 1, 3))

    out = {}
    mk = np.asarray(inputs['att_mix_k'], np.float32).reshape(P, C)
    mv = np.asarray(inputs['att_mix_v'], np.float32).reshape(P, C)
    mr = np.asarray(inputs['att_mix_r'], np.float32).reshape(P, C)
    for p in range(P):
        out[p] = {
            'wk1': q8(fold_pair(inputs['Wk'], mk[p])),
            'wk2': q8(fold_pair(inputs['Wk'], 1 - mk[p])),
            'wv1': q8(fold_pair(inputs['Wv'], mv[p])),
            'wv2': q8(fold_pair(inputs['Wv'], 1 - mv[p])),
            'wr1': q8(fold_pair(inputs['Wr'], mr[p])),
            'wr2': q8(fold_pair(inputs['Wr'], 1 - mr[p])),
        }
    shared = {'wo8': q8(plain_pair(inputs['Wo']))}
    # wck8[hb, p, j, i, dd] = Wck.T[(2j+i)*128+p, hb*128+dd] * WS
    WckT = np.asarray(inputs['Wck'], np.float32).T
    shared['wck8'] = q8(np.ascontiguousarray(
        WckT.reshape(PR, 2, 128, HB, 128).transpose(3, 2, 0, 1, 4)))
    WcvT = np.asarray(inputs['Wcv'], np.float32).T
    shared['wcv8'] = q8(np.ascontiguousarray(
        WcvT.reshape(HPR, 2, 128, CB, 128).transpose(3, 2, 0, 1, 4)))
    WcrT = np.asarray(inputs['Wcr'], np.float32).T
    shared['wcr8'] = q8(np.ascontiguousarray(
        WcrT.reshape(PR, 2, 128, CB, 128).transpose(3, 2, 0, 1, 4)))
    return out, shared


def kernel(**inputs):
    from concourse.bass_utils import run_bass_kernel_spmd

    x = np.asarray(inputs['x'], dtype=np.float32)
    g1 = np.asarray(inputs['ln1_g'], np.float32)
    b1 = np.asarray(inputs['ln1_b'], np.float32)
    g2 = np.asarray(inputs['ln2_g'], np.float32)
    b2 = np.asarray(inputs['ln2_b'], np.float32)
    use_gb1 = not (np.all(g1 == 1.0) and np.all(b1 == 0.0))
    use_gb2 = not (np.all(g2 == 1.0) and np.all(b2 == 0.0))
    cmk = np.asarray(inputs['cm_mix_k'], np.float32).reshape(P, C)
    cmr = np.asarray(inputs['cm_mix_r'], np.float32).reshape(P, C)
    assert np.array_equal(cmk, cmr), "kernel2 assumes cm_mix_k == cm_mix_r"

    debug = os.environ.get('RWKV_DEBUG', '0') == '1'
    key = (use_gb1, use_gb2, debug)
    if key not in _CACHE:
        _CACHE[key] = _build(use_gb1, use_gb2, debug)
    nc = _CACHE[key]

    bf = ml_dtypes.bfloat16
    lam = np.exp(-np.exp(np.asarray(inputs['time_decay'], np.float32)))
    # row 1 is raw u = time_first: it enters as the exp() bias on the device
    u = np.asarray(inputs['time_first'], np.float32)
    vec6 = np.stack([lam.astype(np.float32), u,
                     g1, b1, g2, b2]).astype(np.float32)

    perp, shared = _prep_weights(inputs)

    xf = x.reshape(P * B, T, C)
    in_maps = []
    for core in range(NCORES):
        seqs = [2 * core, 2 * core + 1]
        p = seqs[0] // B
        assert seqs[1] // B == p
        xcm = np.ascontiguousarray(xf[seqs].transpose(0, 2, 1)).astype(bf)
        in_maps.append({
            'xcm': xcm, 'vec6': vec6,
            'mixcm': np.stack([cmk[p], cmk[p]]).astype(np.float32),
            **perp[p], **shared,
        })

    trace = os.environ.get('RWKV_TRACE', '0') == '1'
    res = run_bass_kernel_spmd(nc, in_maps, list(range(NCORES)), trace=trace)
    global LAST_RUN_INFO
    LAST_RUN_INFO = res

    out = np.empty((P * B, T, C), np.float32)
    for core in range(NCORES):
        oc = res.results[core]['oct']
        out[2 * core] = oc[0].T
        out[2 * core + 1] = oc[1].T
    return out.reshape(P, B, T, C)


LAST_RUN_INFO = None


# revision 7
# speedup vs baseline: 1.1287x; 1.0216x over previous
# RWKV-v4 block (TimeMix WKV + ChannelMix) on 8 Trainium2 NeuronCores — v2.
#
# Sharding: data-parallel over the 16 (p, b) sequences -> 2 per core (both
# sequences of a core share the same p, so mix/fold constants are per-core).
#
# v2 strategy vs the bf16 baseline (924us cost-model):
# - All seven matmuls run as fp8e4m3 DoubleRow (2 k-tiles interleaved,
#   K=256/instr at 0.5 cyc/row): 4x fewer PE cycles than bf16.
# - TimeMix token-shift mixes are folded into the weights:
#   k = (Wk*diag(mk)) @ h + (Wk*diag(1-mk)) @ shift(h); h itself is written
#   directly in fp8 pair tiles [128, 2, TC+1] (col 0 = carry), so no mix
#   tiles and no shift ops exist on the vector engines for the k/v/r paths.
# - Weights are host-prescaled by WS=64 into the e4m3 normal range; every
#   inverse scale folds into an Activation scale= or the sigmoid chain.
# - WKV pipeline in bf16 (2x DVE mode on tensor_tensor), scans keep f32
#   lambda, reciprocals in f32 (hw requirement).
# - ChannelMix: cm_mix_k == cm_mix_r (runtime-checked) -> one shared mix
#   tile in fp8; Wck/Wcv are streamed per-unit in hb-/cb-blocked fp8 pair
#   layouts; relu on Act, squares split Pool/DVE -> fp8 (splitting the
#   32-op serial run across engines shortens the per-unit critical path).
import os
import numpy as np
import ml_dtypes

P, B, T, C = 2, 8, 1024, 1024
H = 4 * C
NCORES = 8
NSEQ = 2
TC = 512
NCH = T // TC      # 2
CB = C // 128      # 8
PR = CB // 2       # 4 channel-block pairs
HB = H // 128      # 32
HPR = HB // 2      # 16
EPS = 1e-5
WS = 64.0          # fp8 weight prescale

_CACHE = {}
PHASES = []


def _build(use_gb1, use_gb2, debug=False):
    import concourse.bass as bass
    import concourse.tile as tile
    from concourse import bacc, mybir

    f32 = mybir.dt.float32
    f32r = mybir.dt.float32r
    bf16 = mybir.dt.bfloat16
    f8 = mybir.dt.float8e4
    AL = mybir.AluOpType
    AF = mybir.ActivationFunctionType
    PM = mybir.MatmulPerfMode

    nc = bacc.Bacc()
    global PHASES
    PHASES = []

    def mark(label):
        # snapshot the next instruction index for phase attribution
        nm = nc.get_next_instruction_name()
        PHASES.append((int(nm.split('-')[1]), label))

    xcm = nc.dram_tensor("xcm", (NSEQ, C, T), bf16, kind="ExternalInput")
    # folded TimeMix weights: [PR, 128, 2, C] each
    wk1 = nc.dram_tensor("wk1", (PR, 128, 2, C), f8, kind="ExternalInput")
    wk2 = nc.dram_tensor("wk2", (PR, 128, 2, C), f8, kind="ExternalInput")
    wv1 = nc.dram_tensor("wv1", (PR, 128, 2, C), f8, kind="ExternalInput")
    wv2 = nc.dram_tensor("wv2", (PR, 128, 2, C), f8, kind="ExternalInput")
    wr1 = nc.dram_tensor("wr1", (PR, 128, 2, C), f8, kind="ExternalInput")
    wr2 = nc.dram_tensor("wr2", (PR, 128, 2, C), f8, kind="ExternalInput")
    wo8 = nc.dram_tensor("wo8", (PR, 128, 2, C), f8, kind="ExternalInput")
    wck8 = nc.dram_tensor("wck8", (HB, 128, PR, 2, 128), f8, kind="ExternalInput")
    wcv8 = nc.dram_tensor("wcv8", (CB, 128, HPR, 2, 128), f8, kind="ExternalInput")
    wcr8 = nc.dram_tensor("wcr8", (CB, 128, PR, 2, 128), f8, kind="ExternalInput")
    vec6 = nc.dram_tensor("vec6", (6, C), f32, kind="ExternalInput")
    mixcm = nc.dram_tensor("mixcm", (NSEQ, C), f32, kind="ExternalInput")
    oct_ = nc.dram_tensor("oct", (NSEQ, C, T), f32, kind="ExternalOutput")
    dbg = {}
    if debug:
        dbg['h'] = nc.dram_tensor("dbg_h", (NCH, PR, 128, 2, TC + 1), f8,
                                  kind="ExternalOutput")
        for n in ('k', 'v', 'r'):
            dbg[n] = nc.dram_tensor(f"dbg_{n}", (NCH, CB, 128, TC), f32,
                                    kind="ExternalOutput")
        dbg['sry'] = nc.dram_tensor("dbg_sry", (NCH, PR, 128, 2, TC), f8,
                                    kind="ExternalOutput")
        dbg['x2'] = nc.dram_tensor("dbg_x2", (NCH, PR, 128, 2, TC), bf16,
                                   kind="ExternalOutput")
        dbg['mix'] = nc.dram_tensor("dbg_mix", (NCH, PR, 128, 2, TC), f8,
                                    kind="ExternalOutput")
        dbg['kk'] = nc.dram_tensor("dbg_kk", (NCH, HPR, 128, 2, TC), f8,
                                   kind="ExternalOutput")
        dbg['num'] = nc.dram_tensor("dbg_num", (NCH, CB, 128, TC), f32,
                                    kind="ExternalOutput")
        dbg['den'] = nc.dram_tensor("dbg_den", (NCH, CB, 128, TC), f32,
                                    kind="ExternalOutput")

    from contextlib import ExitStack
    with ExitStack() as ctx:
        tc = ctx.enter_context(tile.TileContext(nc))
        pc = ctx.enter_context(tc.tile_pool(name="const", bufs=1))
        pw = ctx.enter_context(tc.tile_pool(name="wres", bufs=1))
        pwck = ctx.enter_context(tc.tile_pool(name="wckst", bufs=6))
        pwcv = ctx.enter_context(tc.tile_pool(name="wcvst", bufs=2))
        px = ctx.enter_context(tc.tile_pool(name="x", bufs=8))
        psq = ctx.enter_context(tc.tile_pool(name="sq", bufs=2))
        ph = ctx.enter_context(tc.tile_pool(name="h", bufs=8))
        pd = ctx.enter_context(tc.tile_pool(name="d", bufs=5))
        pstat = ctx.enter_context(tc.tile_pool(name="stat", bufs=2))
        pbc = ctx.enter_context(tc.tile_pool(name="bcc", bufs=4))
        pg = ctx.enter_context(tc.tile_pool(name="gen", bufs=12))
        pf32 = ctx.enter_context(tc.tile_pool(name="f32s", bufs=5))
        pga = ctx.enter_context(tc.tile_pool(name="genA", bufs=6))
        psry = ctx.enter_context(tc.tile_pool(name="sry", bufs=4))
        px2 = ctx.enter_context(tc.tile_pool(name="x2", bufs=6))
        ph2 = ctx.enter_context(tc.tile_pool(name="h2", bufs=8, ))
        pmix = ctx.enter_context(tc.tile_pool(name="mix", bufs=8))
        prelu = ctx.enter_context(tc.tile_pool(name="relu", bufs=3))
        pkk = ctx.enter_context(tc.tile_pool(name="kk", bufs=18))
        pout = ctx.enter_context(tc.tile_pool(name="out", bufs=2))
        psmm = ctx.enter_context(tc.tile_pool(name="psmm", bufs=6, space="PSUM"))
        psst = ctx.enter_context(tc.tile_pool(name="pss", bufs=2, space="PSUM"))

        # ---- constants ----
        invCb = pc.tile([128, 1], bf16, tag="invCb")
        nc.vector.memset(invCb[:], 1.0 / C)
        eps_t = pc.tile([128, 1], f32, tag="eps")
        nc.vector.memset(eps_t[:], EPS)
        negone_f = pc.tile([1, 1], f32, tag="negonef")
        nc.vector.memset(negone_f[:], -1.0)
        negone = pc.tile([1, 1], f32r, tag="negone")
        nc.scalar.copy(negone[:], negone_f[:])

        def colload(src_ap, ncol, dtype=f32, tag=None):
            t = pc.tile([128, ncol], dtype, tag=tag)
            nc.sync.dma_start(t[:], src_ap)
            return t

        lam_c = colload(vec6[0].rearrange("(j p) -> p j", p=128), CB, tag="lam")
        eu_c = colload(vec6[1].rearrange("(j p) -> p j", p=128), CB, tag="eu")
        g1_c = colload(vec6[2].rearrange("(j p) -> p j", p=128), CB, tag="g1")
        b1_c = colload(vec6[3].rearrange("(j p) -> p j", p=128), CB, tag="b1")
        g2_c = colload(vec6[4].rearrange("(j p) -> p j", p=128), CB, tag="g2")
        b2_c = colload(vec6[5].rearrange("(j p) -> p j", p=128), CB, tag="b2")
        cmk_c = colload(mixcm.rearrange("s (j p) -> p (s j)", p=128),
                        NSEQ * CB, tag="cmk")

        # carries (chunk -> chunk)
        carryH = pc.tile([128, NSEQ * CB], f8, tag="carryH")     # h pairs
        carryH2 = pc.tile([128, NSEQ * CB], bf16, tag="carryH2")  # h2
        carryA = pc.tile([128, NSEQ * CB], bf16, tag="carryA")
        carryB = pc.tile([128, NSEQ * CB], bf16, tag="carryB")

        # ---- resident weights ----
        def wload(src, tag):
            tiles = []
            for j in range(PR):
                t = pw.tile([128, 2, C], f8, tag=f"{tag}{j}")
                nc.sync.dma_start(t[:], src[j])
                tiles.append(t)
            return tiles

        wk1_sb, wk2_sb, wv1_sb, wv2_sb = [], [], [], []
        wr1_sb, wr2_sb, wo_sb, wcr_sb = [], [], [], []

        def load_weights():
            # emitted AFTER unit 0's tm1 so the first x DMAs are not queued
            # behind 7MB of weight traffic on the SP/HWDGE path
            for lst, src, tag in ((wk1_sb, wk1, "wk1"), (wk2_sb, wk2, "wk2"),
                                  (wv1_sb, wv1, "wv1"), (wv2_sb, wv2, "wv2"),
                                  (wr1_sb, wr1, "wr1"), (wr2_sb, wr2, "wr2"),
                                  (wo_sb, wo8, "wo")):
                lst.extend(wload(src, tag))
            for cb in range(CB):
                t = pw.tile([128, PR, 2, 128], f8, tag=f"wcr{cb}",
                            name=f"wcr{cb}")
                nc.sync.dma_start(t[:], wcr8[cb])
                wcr_sb.append(t)

        def ln_begin():
            s1 = psst.tile([1, TC], f32, tag="ss", name="s1")
            s2 = psst.tile([1, TC], f32, tag="ss", name="s2")
            return s1, s2

        def ln_feed(st, pair, j):
            """accumulate pair j's s1/s2 contributions (PE + one DVE sq)"""
            s1, s2 = st
            for i in range(2):
                nc.tensor.matmul(s1[:], invCb[:], pair[:, i, :],
                                 start=(j == 0 and i == 0),
                                 stop=(j == PR - 1 and i == 1))
                sq = psq.tile([128, TC], bf16, tag="sq")
                nc.vector.tensor_tensor(sq[:], pair[:, i, :],
                                        pair[:, i, :], AL.mult)
                nc.tensor.matmul(s2[:], invCb[:], sq[:],
                                 start=(j == 0 and i == 0), stop=False)

        def ln_finish(st):
            """var = s2 - mu^2 via an extra PSUM accumulation matmul so the
            stats chain stays PE<->Act only; returns broadcast (mu_sb, rs_sb)."""
            s1, s2 = st
            mu2 = pstat.tile([1, TC], f32r, tag="st")
            nc.scalar.activation(mu2[:], s1[:], AF.Square)
            mub = pstat.tile([1, TC], bf16, tag="st")
            nc.scalar.activation(mub[:], s1[:], AF.Copy)
            nc.tensor.matmul(s2[:], negone[:], mu2[:], start=False, stop=True)
            lnv = pstat.tile([1, TC], f32, tag="st")
            nc.scalar.activation(lnv[:], s2[:], AF.Ln, bias=eps_t[0:1, 0:1])
            rs = pstat.tile([1, TC], bf16, tag="st")
            nc.scalar.activation(rs[:], lnv[:], AF.Exp, scale=-0.5)
            mu_sb = pbc.tile([128, TC], bf16, tag="mu_sb")
            nc.gpsimd.partition_broadcast(mu_sb[:], mub[:], 128)
            rs_sb = pbc.tile([128, TC], bf16, tag="rs_sb")
            nc.gpsimd.partition_broadcast(rs_sb[:], rs[:], 128)
            return mu_sb, rs_sb

        def ln_stats(src_pairs, g_c, b_c, use_gb):
            st = ln_begin()
            for j in range(PR):
                ln_feed(st, src_pairs[j], j)
            return ln_finish(st)

        def tm1_stats(s, ch):
            xts = []
            xsrc = xcm[s].rearrange("(j i p) t -> j p i t", i=2, p=128)
            for j in range(PR):
                xt = px.tile([128, 2, TC], bf16, tag="x")
                nc.sync.dma_start(xt[:], xsrc[j, :, :, ch * TC:(ch + 1) * TC])
                xts.append(xt)
            mu_sb, rs_sb = ln_stats(xts, g1_c, b1_c, use_gb1)
            return xts, mu_sb, rs_sb

        def tm1_apply(s, ch, st):
            xts, mu_sb, rs_sb = st
            hts = []
            for j in range(PR):
                ht = ph.tile([128, 2, TC + 1], f8, tag="h")
                for i in range(2):
                    cb = 2 * j + i
                    t = pd.tile([128, TC], bf16, tag="d")
                    nc.vector.tensor_tensor(t[:], xts[j][:, i, :], mu_sb[:],
                                            AL.subtract)
                    if use_gb1:
                        tg = pd.tile([128, TC], bf16, tag="d")
                        nc.vector.tensor_tensor(tg[:], t[:], rs_sb[:], AL.mult)
                        nc.vector.tensor_scalar(
                            ht[:, i, 1:TC + 1], tg[:], g_col(g1_c, cb),
                            b_col(b1_c, cb), AL.mult, AL.add)
                    else:
                        nc.vector.tensor_tensor(ht[:, i, 1:TC + 1], t[:],
                                                rs_sb[:], AL.mult)
                idx = s * CB + 2 * j
                if ch == 0:
                    nc.vector.memset(ht[:, :, 0:1], 0.0)
                else:
                    nc.vector.tensor_copy(ht[:, :, 0:1], carryH[:, idx:idx + 2])
                if ch < NCH - 1:
                    nc.vector.tensor_copy(carryH[:, idx:idx + 2],
                                          ht[:, :, TC:TC + 1])
                if debug and s == 0:
                    nc.sync.dma_start(dbg['h'][ch, j], ht[:])
                hts.append(ht)
            return hts

        def g_col(gc, cb):
            return gc[:, cb:cb + 1]

        def mm8(psum, w1_sb, w2_sb, hts, db):
            """accumulate W1 @ h + W2 @ shift(h) into psum (8 DR matmuls)"""
            lo, hi = db * 128, (db + 1) * 128
            for j in range(PR):
                nc.tensor.matmul(psum[:], w1_sb[j][:, :, lo:hi],
                                 hts[j][:, :, 1:TC + 1],
                                 start=(j == 0), stop=False,
                                 perf_mode=PM.DoubleRow)
            for j in range(PR):
                nc.tensor.matmul(psum[:], w2_sb[j][:, :, lo:hi],
                                 hts[j][:, :, 0:TC],
                                 start=False, stop=(j == PR - 1),
                                 perf_mode=PM.DoubleRow)

        def tm2(s, ch, xts, hts):
            sry_ts = [psry.tile([128, 2, TC], f8, tag="sry", name=f"sry{_j}")
                      for _j in range(PR)]
            for db in range(CB):
                idx = s * CB + db
                kps = psmm.tile([128, TC], f32, tag="mm")
                mm8(kps, wk1_sb, wk2_sb, hts, db)
                vps = psmm.tile([128, TC], f32, tag="mm")
                mm8(vps, wv1_sb, wv2_sb, hts, db)
                rps = psmm.tile([128, TC], f32, tag="mm")
                mm8(rps, wr1_sb, wr2_sb, hts, db)
                if debug and s == 0:
                    for nme, ps in (('k', kps), ('v', vps), ('r', rps)):
                        dtmp = pd.tile([128, TC], f32, tag="d",
                                       name=f"dbg{nme}{db}")
                        nc.scalar.activation(dtmp[:], ps[:], AF.Copy,
                                             scale=1.0 / WS)
                        nc.sync.dma_start(dbg[nme][ch, db], dtmp[:])

                ek = pg.tile([128, TC], bf16, tag="gen")
                nc.scalar.activation(ek[:], kps[:], AF.Exp, scale=1.0 / WS)
                eku = pg.tile([128, TC], bf16, tag="gen")
                nc.scalar.activation(eku[:], kps[:], AF.Exp, scale=1.0 / WS,
                                     bias=eu_c[:, db:db + 1])
                vbf = pg.tile([128, TC], bf16, tag="gen")
                nc.scalar.activation(vbf[:], vps[:], AF.Copy, scale=1.0 / WS)
                enr = pg.tile([128, TC], bf16, tag="gen")
                nc.scalar.activation(enr[:], rps[:], AF.Exp, scale=-1.0 / WS)

                ekv = pg.tile([128, TC], bf16, tag="gen")
                nc.gpsimd.tensor_tensor(ekv[:], ek[:], vbf[:], AL.mult)
                ekuv = pg.tile([128, TC], bf16, tag="gen")
                nc.gpsimd.tensor_tensor(ekuv[:], eku[:], vbf[:], AL.mult)

                At = pga.tile([128, TC + 1], bf16, tag="genA")
                Bt = pga.tile([128, TC + 1], bf16, tag="genA")
                if ch == 0:
                    nc.vector.memset(At[:, 0:1], 0.0)
                    nc.vector.memset(Bt[:, 0:1], 0.0)
                else:
                    nc.vector.tensor_copy(At[:, 0:1], carryA[:, idx:idx + 1])
                    nc.vector.tensor_copy(Bt[:, 0:1], carryB[:, idx:idx + 1])
                lamb = lam_c[:, db:db + 1].broadcast_to((128, TC))
                nc.vector.tensor_tensor_scan(At[:, 1:TC + 1], lamb, ekv[:],
                                             At[:, 0:1], AL.mult, AL.add)
                nc.vector.tensor_tensor_scan(Bt[:, 1:TC + 1], lamb, ek[:],
                                             Bt[:, 0:1], AL.mult, AL.add)
                if ch < NCH - 1:
                    nc.vector.tensor_copy(carryA[:, idx:idx + 1],
                                          At[:, TC:TC + 1])
                    nc.vector.tensor_copy(carryB[:, idx:idx + 1],
                                          Bt[:, TC:TC + 1])

                num = pg.tile([128, TC], bf16, tag="gen")
                nc.vector.tensor_tensor(num[:], At[:, 0:TC], ekuv[:], AL.add)
                den = pg.tile([128, TC], bf16, tag="gen")
                nc.vector.tensor_tensor(den[:], Bt[:, 0:TC], eku[:], AL.add)
                enr1 = pg.tile([128, TC], bf16, tag="gen")
                nc.vector.tensor_scalar_add(enr1[:], enr[:], 1.0)
                den2 = pf32.tile([128, TC], f32, tag="f32s")
                nc.vector.tensor_tensor(den2[:], den[:], enr1[:], AL.mult)
                rec = pf32.tile([128, TC], f32, tag="f32s")
                nc.vector.reciprocal_approx_fast(rec[:], den2[:])
                nc.vector.tensor_tensor(sry_ts[db // 2][:, db % 2, :],
                                        num[:], rec[:], AL.mult)
                if debug and s == 0:
                    for nme, src in (('num', num), ('den', den)):
                        dtmp2 = pd.tile([128, TC], f32, tag="d",
                                        name=f"dbg{nme}{db}")
                        nc.vector.tensor_copy(dtmp2[:], src[:])
                        nc.sync.dma_start(dbg[nme][ch, db], dtmp2[:])

            x2_ts = []
            for j in range(PR):
                x2t = px2.tile([128, 2, TC], bf16, tag="x2")
                for i in range(2):
                    cb = 2 * j + i
                    xps = psmm.tile([128, TC], f32, tag="mm")
                    for jj in range(PR):
                        nc.tensor.matmul(
                            xps[:], wo_sb[jj][:, :, cb * 128:(cb + 1) * 128],
                            sry_ts[jj][:], start=(jj == 0), stop=(jj == PR - 1),
                            perf_mode=PM.DoubleRow)
                    xevac = pd.tile([128, TC], bf16, tag="d")
                    nc.scalar.activation(xevac[:], xps[:], AF.Copy,
                                         scale=1.0 / WS)
                    nc.vector.tensor_tensor(x2t[:, i, :], xevac[:],
                                            xts[j][:, i, :], AL.add)
                if debug and s == 0:
                    nc.sync.dma_start(dbg['sry'][ch, j], sry_ts[j][:])
                    nc.sync.dma_start(dbg['x2'][ch, j], x2t[:])
                x2_ts.append(x2t)
            return x2_ts

        def cm1(s, ch, x2_ts):
            mu_sb, rs_sb = ln_stats(x2_ts, g2_c, b2_c, use_gb2)
            h2ts = []
            for cb in range(CB):
                j, i = cb // 2, cb % 2
                h2t = ph2.tile([128, TC + 1], bf16, tag="h2")
                t = pd.tile([128, TC], bf16, tag="d")
                nc.vector.tensor_tensor(t[:], x2_ts[j][:, i, :], mu_sb[:],
                                        AL.subtract)
                nc.vector.tensor_tensor(h2t[:, 1:TC + 1], t[:], rs_sb[:],
                                        AL.mult)
                if use_gb2:
                    nc.vector.tensor_scalar(h2t[:, 1:TC + 1], h2t[:, 1:TC + 1],
                                            g_col(g2_c, cb), g_col(b2_c, cb),
                                            AL.mult, AL.add)
                idx = s * CB + cb
                if ch == 0:
                    nc.vector.memset(h2t[:, 0:1], 0.0)
                else:
                    nc.vector.tensor_copy(h2t[:, 0:1], carryH2[:, idx:idx + 1])
                if ch < NCH - 1:
                    nc.vector.tensor_copy(carryH2[:, idx:idx + 1],
                                          h2t[:, TC:TC + 1])
                h2ts.append(h2t)
            mix_ts = [pmix.tile([128, 2, TC], f8, tag="mix", name=f"mix{_j}")
                      for _j in range(PR)]
            for cb in range(CB):
                idx = s * CB + cb
                dt = pd.tile([128, TC], bf16, tag="d")
                nc.vector.tensor_tensor(dt[:], h2ts[cb][:, 1:TC + 1],
                                        h2ts[cb][:, 0:TC], AL.subtract)
                nc.vector.scalar_tensor_tensor(
                    mix_ts[cb // 2][:, cb % 2, :], dt[:],
                    cmk_c[:, idx:idx + 1], h2ts[cb][:, 0:TC], AL.mult, AL.add)
            kk_ts = [pkk.tile([128, 2, TC], f8, tag="kk", name=f"kk{_j}")
                     for _j in range(HPR)]
            for hb in range(HB):
                wckt = pwck.tile([128, PR, 2, 128], f8, tag="wck")
                nc.sync.dma_start(wckt[:], wck8[hb])
                ckps = psmm.tile([128, TC], f32, tag="mm")
                for j in range(PR):
                    nc.tensor.matmul(ckps[:], wckt[:, j], mix_ts[j][:],
                                     start=(j == 0), stop=(j == PR - 1),
                                     perf_mode=PM.DoubleRow)
                relu = prelu.tile([128, TC], bf16, tag="relu")
                nc.scalar.activation(relu[:], ckps[:], AF.Relu, scale=1.0 / WS)
                if hb % 2 == 0:
                    nc.gpsimd.tensor_tensor(kk_ts[hb // 2][:, hb % 2, :],
                                            relu[:], relu[:], AL.mult)
                else:
                    nc.vector.tensor_tensor(kk_ts[hb // 2][:, hb % 2, :],
                                            relu[:], relu[:], AL.mult)
            if debug and s == 0:
                for j in range(PR):
                    nc.sync.dma_start(dbg['mix'][ch, j], mix_ts[j][:])
                for j in range(HPR):
                    nc.sync.dma_start(dbg['kk'][ch, j], kk_ts[j][:])
            return x2_ts, mix_ts, kk_ts

        def cm2(s, ch, st3):
            x2_ts, mix_ts, kk_ts = st3
            for cb in range(CB):
                crps = psmm.tile([128, TC], f32, tag="mm")
                for j in range(PR):
                    nc.tensor.matmul(crps[:], wcr_sb[cb][:, j], mix_ts[j][:],
                                     start=(j == 0), stop=(j == PR - 1),
                                     perf_mode=PM.DoubleRow)
                enz = pg.tile([128, TC], bf16, tag="gen")
                nc.scalar.activation(enz[:], crps[:], AF.Exp, scale=-1.0 / WS)
                enz1 = pf32.tile([128, TC], f32, tag="f32s")
                nc.vector.tensor_scalar(enz1[:], enz[:], WS, WS,
                                        AL.mult, AL.add)
                rec = pf32.tile([128, TC], f32, tag="f32s")
                nc.vector.reciprocal_approx_fast(rec[:], enz1[:])
                wcvt = pwcv.tile([128, HPR, 2, 128], f8, tag="wcv")
                nc.sync.dma_start(wcvt[:], wcv8[cb])
                kvps = psmm.tile([128, TC], f32, tag="mm")
                for j in range(HPR):
                    nc.tensor.matmul(kvps[:], wcvt[:, j], kk_ts[j][:],
                                     start=(j == 0), stop=(j == HPR - 1),
                                     perf_mode=PM.DoubleRow)
                t1 = pd.tile([128, TC], bf16, tag="d")
                nc.vector.tensor_tensor(t1[:], kvps[:], rec[:], AL.mult)
                outt = pout.tile([128, TC], f32, tag="out")
                nc.vector.tensor_tensor(outt[:], x2_ts[cb // 2][:, cb % 2, :],
                                        t1[:], AL.add)
                nc.sync.dma_start(
                    oct_[s, cb * 128:(cb + 1) * 128, ch * TC:(ch + 1) * TC],
                    outt[:])

        # software-pipelined emission as in the baseline:
        # tm1(u) -> cm2(prev) -> tm2(u) -> cm1(u)
        # ch-major: consecutive units belong to different sequences, so
        # their chains are independent and overlap on every engine.
        units = [(s, ch) for ch in range(NCH) for s in range(NSEQ)]
        cm1_st = {}
        prev = None
        u0 = units[0]
        mark(f"tm1{u0}")
        st0 = tm1_stats(*u0)
        hts0 = tm1_apply(*u0, st0)
        load_weights()
        for idx, u in enumerate(units):
            if idx == 0:
                st_u, hts_u = st0, hts0
            else:
                mark(f"tm1{u}")
                st_u = tm1_stats(*u)
                hts_u = tm1_apply(*u, st_u)
            if prev is not None:
                mark(f"cm2{prev}")
                cm2(*prev, cm1_st.pop(prev))
            mark(f"tm2{u}")
            x2_ts = tm2(*u, st_u[0], hts_u)
            mark(f"cm1{u}")
            cm1_st[u] = cm1(*u, x2_ts)
            prev = u
        mark(f"cm2{prev}")
        cm2(*prev, cm1_st.pop(prev))
        mark("end")

    nc.compile()
    return nc


def _prep_weights(inputs):
    """Host-side fp8 weight prep. Returns dict of arrays shared by all cores
    plus per-p folded TimeMix weights."""
    bf = ml_dtypes.bfloat16
    f8 = ml_dtypes.float8_e4m3

    def q8(a):
        return np.clip(a * WS, -240, 240).astype(f8)

    def fold_pair(W, m):
        # lhsT[c, d] = W[d, c] * m[c]; layout [j, p, i, d]
        WT = np.asarray(W, np.float32).T * m[:, None]
        return np.ascontiguousarray(
            WT.reshape(PR, 2, 128, C).transpose(0, 2, 1, 3))

    def plain_pair(W):
        WT = np.ascontiguousarray(np.asarray(W, np.float32).T)
        return np.ascontiguousarray(
            WT.reshape(PR, 2, 128, C).transpose(0, 2, 1, 3))

    out = {}
    mk = np.asarray(inputs['att_mix_k'], np.float32).reshape(P, C)
    mv = np.asarray(inputs['att_mix_v'], np.float32).reshape(P, C)
    mr = np.asarray(inputs['att_mix_r'], np.float32).reshape(P, C)
    for p in range(P):
        out[p] = {
            'wk1': q8(fold_pair(inputs['Wk'], mk[p])),
            'wk2': q8(fold_pair(inputs['Wk'], 1 - mk[p])),
            'wv1': q8(fold_pair(inputs['Wv'], mv[p])),
            'wv2': q8(fold_pair(inputs['Wv'], 1 - mv[p])),
            'wr1': q8(fold_pair(inputs['Wr'], mr[p])),
            'wr2': q8(fold_pair(inputs['Wr'], 1 - mr[p])),
        }
    shared = {'wo8': q8(plain_pair(inputs['Wo']))}
    # wck8[hb, p, j, i, dd] = Wck.T[(2j+i)*128+p, hb*128+dd] * WS
    WckT = np.asarray(inputs['Wck'], np.float32).T
    shared['wck8'] = q8(np.ascontiguousarray(
        WckT.reshape(PR, 2, 128, HB, 128).transpose(3, 2, 0, 1, 4)))
    WcvT = np.asarray(inputs['Wcv'], np.float32).T
    shared['wcv8'] = q8(np.ascontiguousarray(
        WcvT.reshape(HPR, 2, 128, CB, 128).transpose(3, 2, 0, 1, 4)))
    WcrT = np.asarray(inputs['Wcr'], np.float32).T
    shared['wcr8'] = q8(np.ascontiguousarray(
        WcrT.reshape(PR, 2, 128, CB, 128).transpose(3, 2, 0, 1, 4)))
    return out, shared


def kernel(**inputs):
    from concourse.bass_utils import run_bass_kernel_spmd

    x = np.asarray(inputs['x'], dtype=np.float32)
    g1 = np.asarray(inputs['ln1_g'], np.float32)
    b1 = np.asarray(inputs['ln1_b'], np.float32)
    g2 = np.asarray(inputs['ln2_g'], np.float32)
    b2 = np.asarray(inputs['ln2_b'], np.float32)
    use_gb1 = not (np.all(g1 == 1.0) and np.all(b1 == 0.0))
    use_gb2 = not (np.all(g2 == 1.0) and np.all(b2 == 0.0))
    cmk = np.asarray(inputs['cm_mix_k'], np.float32).reshape(P, C)
    cmr = np.asarray(inputs['cm_mix_r'], np.float32).reshape(P, C)
    assert np.array_equal(cmk, cmr), "kernel2 assumes cm_mix_k == cm_mix_r"

    debug = os.environ.get('RWKV_DEBUG', '0') == '1'
    key = (use_gb1, use_gb2, debug)
    if key not in _CACHE:
        _CACHE[key] = _build(use_gb1, use_gb2, debug)
    nc = _CACHE[key]

    bf = ml_dtypes.bfloat16
    lam = np.exp(-np.exp(np.asarray(inputs['time_decay'], np.float32)))
    # row 1 is raw u = time_first: it enters as the exp() bias on the device
    u = np.asarray(inputs['time_first'], np.float32)
    vec6 = np.stack([lam.astype(np.float32), u,
                     g1, b1, g2, b2]).astype(np.float32)

    perp, shared = _prep_weights(inputs)

    xf = x.reshape(P * B, T, C)
    in_maps = []
    for core in range(NCORES):
        seqs = [2 * core, 2 * core + 1]
        p = seqs[0] // B
        assert seqs[1] // B == p
        xcm = np.ascontiguousarray(xf[seqs].transpose(0, 2, 1)).astype(bf)
        in_maps.append({
            'xcm': xcm, 'vec6': vec6,
            'mixcm': np.stack([cmk[p], cmk[p]]).astype(np.float32),
            **perp[p], **shared,
        })

    trace = os.environ.get('RWKV_TRACE', '0') == '1'
    res = run_bass_kernel_spmd(nc, in_maps, list(range(NCORES)), trace=trace)
    global LAST_RUN_INFO
    LAST_RUN_INFO = res

    out = np.empty((P * B, T, C), np.float32)
    for core in range(NCORES):
        oc = res.results[core]['oct']
        out[2 * core] = oc[0].T
        out[2 * core + 1] = oc[1].T
    return out.reshape(P, B, T, C)


LAST_RUN_INFO = None
